# revision 1
# baseline (speedup 1.0000x reference)
"""DifferentialGPT forward on 8 TRN2 NeuronCores (Bass/Tile).

DP2 over batch x TP4 (4 heads, 512 MLP pairs, 12800 vocab cols per core);
AllReduce in groups of 4 after out_proj and c_proj.
Residual kept transposed xT [C(part), T(free)]; matmuls in float32r.
"""
import math
import numpy as np

V, BLK, C, H, L = 50257, 1024, 1024, 16, 4
B, T = 2, 1024
HS = C // H          # 64
D = HS // 2          # 32
EPS = 1e-5
N_CORES = 8
TP = 4
HPC = H // TP        # 4
VSH = 12800
HIDS = 4 * C // TP // 2   # 512
NCB = C // 128       # 8
NT = T // 512        # 2
NSB = T // 128       # 8

_BUILT = {}


def _build(rep_count=1):
    from concourse import bass, mybir, bacc
    import concourse.tile as tile

    F32 = mybir.dt.float32
    F32R = mybir.dt.float32r
    BF16 = mybir.dt.bfloat16
    AF = mybir.ActivationFunctionType
    ALU = mybir.AluOpType

    nc = bacc.Bacc("TRN2", target_bir_lowering=False, debug=False,
                   num_devices=N_CORES)
    for _cv in (EPS,):
        _ct = nc.alloc_sbuf_tensor(f"const-float32-{_cv}", [128, 1], F32)
        nc.gpsimd.memset(_ct.ap(), _cv)
        nc.const_aps.aps[(F32, _cv)] = _ct.ap()
    nc.all_engine_barrier()

    def EIN(name, shape, dt):
        return nc.dram_tensor(name, shape, dt, kind="ExternalInput")

    oneh_e = EIN("oneh", [BLK, T], F32R)
    wteE_e = EIN("wteE", [BLK, C], F32R)
    wq_e = EIN("wq", [L, C, HPC * HS], F32R)
    wk_e = EIN("wk", [L, C, HPC * HS], F32R)
    wv_e = EIN("wv", [L, C, HPC * HS], F32R)
    ow_e = EIN("ow", [L, HPC * HS, C], F32R)
    w1_e = EIN("w1", [L, C, 2 * HIDS], F32R)
    w2_e = EIN("w2", [L, HIDS, C], F32R)
    neglam_e = EIN("neglam", [L, HPC], F32)
    masks_e = EIN("masks", [128, 4, 2048], BF16)
    wteT_e = EIN("wteT", [C, VSH], F32R)
    logits_e = nc.dram_tensor("logits", [T, VSH], F32, kind="ExternalOutput")

    GROUPS = [[0, 1, 2, 3], [4, 5, 6, 7]]
    holder = {}

    def bcast_row(sp, at, row_ap, width, tag, recip=False):
        ones_row = holder["ones_row"]
        bp = sp.tile([128, width], F32, tag=f"bp_{tag}", name=f"bp_{tag}")
        for tt in range(width // 512):
            nc.tensor.matmul(bp[:, tt * 512:(tt + 1) * 512], ones_row[:],
                             row_ap[0:1, tt * 512:(tt + 1) * 512],
                             start=True, stop=True)
        out = at.tile([128, width], F32, tag=f"bo_{tag}", name=f"bo_{tag}")
        if recip:
            for tt in range(width // 512):
                nc.vector.reciprocal_approx_fast(
                    out=out[:, tt * 512:(tt + 1) * 512],
                    in_=bp[:, tt * 512:(tt + 1) * 512])
        else:
            nc.scalar.copy(out[:], bp[:])
        return out

    def rowstats(sp, src, tag, lhs):
        ps = sp.tile([1, T], F32, tag=tag, name=tag)
        for tt in range(NT):
            for cb in range(NCB):
                nc.tensor.matmul(
                    ps[:, tt * 512:(tt + 1) * 512], lhs,
                    src[:, cb, tt * 512:(tt + 1) * 512],
                    start=(cb == 0), stop=(cb == NCB - 1))
        return ps

    with tile.TileContext(nc) as tc:
      with (
        tc.tile_pool(name="persist", bufs=1) as pp,
        tc.tile_pool(name="dram", bufs=1, space="DRAM") as dram,
      ):
        masks = pp.tile([128, 4, 2048], BF16, name="masks")
        nc.sync.dma_start(out=masks[:], in_=masks_e[:, :, :])
        neglam = pp.tile([128, L, HPC], F32, name="neglam")
        nc.sync.dma_start(
            out=neglam[:],
            in_=bass.AP(tensor=neglam_e, offset=0,
                        ap=[[0, 128], [HPC, L], [1, HPC]]))
        ones_f = pp.tile([128, 128], F32, name="ones_f")
        nc.vector.memset(ones_f[:], 1.0)
        ones_col = pp.tile([128, 1], F32R, name="ones_col")
        nc.vector.tensor_copy(ones_col[:], ones_f[:, 0:1])
        ones_cb2 = pp.tile([128, 2], BF16, name="ones_cb2")
        nc.vector.tensor_copy(ones_cb2[:], ones_f[:, 0:2])
        ones_cb = ones_cb2[:, 0:1]
        ones_row = pp.tile([1, 128], F32R, name="ones_row")
        nc.vector.tensor_copy(ones_row[:], ones_f[0:1, :])
        ones128 = pp.tile([128, 128], F32R, name="ones128")
        nc.vector.tensor_copy(ones128[:], ones_f[:])
        holder["ones_row"] = ones_row

        for _rep in range(rep_count):
          with tc.tile_pool(name=f"xpool{_rep}", bufs=1) as xpool:
            xT = xpool.tile([128, NCB, T], F32R, name="xT")

            # ---------------- embedding ----------------
            with (
                tc.tile_pool(name=f"emb{_rep}", bufs=1) as emb,
                tc.tile_pool(name=f"embp{_rep}", bufs=2, space="PSUM") as embp,
            ):
                wteE = emb.tile([128, BLK // 128, C], F32R, name="wteE")
                nc.sync.dma_start(
                    out=wteE[:],
                    in_=wteE_e.ap().rearrange("(a p) c -> p a c", p=128))
                oneh = emb.tile([128, BLK // 128, T], F32R, name="oneh")
                nc.sync.dma_start(
                    out=oneh[:],
                    in_=oneh_e.ap().rearrange("(a p) t -> p a t", p=128))
                for cb in range(NCB):
                    for tt in range(NT):
                        pe = embp.tile([128, 512], F32, tag="pe", name="pe")
                        for kb in range(BLK // 128):
                            nc.tensor.matmul(
                                pe[:], wteE[:, kb, cb * 128:(cb + 1) * 128],
                                oneh[:, kb, tt * 512:(tt + 1) * 512],
                                start=(kb == 0), stop=(kb == BLK // 128 - 1))
                        nc.scalar.copy(xT[:, cb, tt * 512:(tt + 1) * 512],
                                       pe[:])

            # ---------------- layers ----------------
            for l in range(L):
              # ======== attention ========
              with tc.tile_pool(name=f"at{_rep}_{l}", bufs=1) as at:
                wq = at.tile([128, NCB, HPC * HS], F32R, name=f"wq{l}")
                wk = at.tile([128, NCB, HPC * HS], F32R, name=f"wk{l}")
                wv = at.tile([128, NCB, HPC * HS], F32R, name=f"wv{l}")
                ow = at.tile([128, (HPC * HS) // 128, C], F32R, name=f"ow{l}")
                for t_sb, t_e in ((wq, wq_e), (wk, wk_e), (wv, wv_e),
                                  (ow, ow_e)):
                    nc.sync.dma_start(
                        out=t_sb[:],
                        in_=t_e.ap()[l].rearrange("(a p) m -> p a m", p=128))
                ar_in = dram.tile([C, T], F32, tag="ari", name=f"ari_a{l}")
                ar_out = dram.tile([C, T], F32, tag="aro", name=f"aro_a{l}")

                with tc.tile_pool(name=f"sp1_{_rep}_{l}", bufs=1, space="PSUM") as sp:
                    sq = at.tile([128, NCB, T], BF16, name=f"sq{l}")
                    for cb in range(NCB):
                        nc.scalar.activation(sq[:, cb, :], xT[:, cb, :],
                                             AF.Square)
                    ssq_r = rowstats(sp, sq, "st_r", ones_cb)
                    ssq_c = sp.tile([128, NSB], F32, tag="st_c",
                                    name=f"ssqc{l}")
                    for sb in range(NSB):
                        for cb in range(NCB):
                            nc.tensor.matmul(
                                ssq_c[:, sb:sb + 1],
                                sq[:, cb, sb * 128:(sb + 1) * 128],
                                ones_cb,
                                start=(cb == 0), stop=(cb == NCB - 1))
                    std_r = at.tile([1, T], F32R, name=f"stdr{l}")
                    nc.scalar.activation(std_r[:], ssq_r[:], AF.Sqrt,
                                         scale=1.0 / C, bias=EPS)
                    rstd_b = bcast_row(sp, at, std_r, T, "r1", recip=True)
                    std_c = at.tile([128, NSB], F32, name=f"stdc{l}")
                    nc.scalar.activation(std_c[:], ssq_c[:], AF.Sqrt,
                                         scale=1.0 / C, bias=EPS)
                    rstd_c = at.tile([128, NSB], F32, name=f"rstdc{l}")
                    nc.vector.reciprocal_approx_fast(out=rstd_c[:],
                                                     in_=std_c[:])

                    qT = at.tile([128, 2, T], F32R, name=f"qT{l}")
                    kT = at.tile([128, 2, T], F32R, name=f"kT{l}")
                    vN = at.tile([128, NSB, HPC * (HS + 1)], F32R,
                                 name=f"vN{l}")
                    vN4 = vN[:].rearrange("p s (h e) -> p s h e", e=HS + 1)
                    nc.vector.tensor_copy(vN4[:, :, :, HS:HS + 1],
                                          ones_f[:, 0:NSB * HPC])
                    for dst, w_sb in ((qT, wq), (kT, wk)):
                        for mb in range(2):
                            for tt in range(NT):
                                pq = sp.tile([128, 512], F32, tag="pq",
                                             bufs=2, name="pq")
                                for cb in range(NCB):
                                    nc.tensor.matmul(
                                        pq[:],
                                        w_sb[:, cb, mb * 128:(mb + 1) * 128],
                                        xT[:, cb, tt * 512:(tt + 1) * 512],
                                        start=(cb == 0),
                                        stop=(cb == NCB - 1))
                                nc.vector.tensor_mul(
                                    dst[:, mb, tt * 512:(tt + 1) * 512],
                                    pq[:],
                                    rstd_b[:, tt * 512:(tt + 1) * 512])
                    for sb in range(NSB):
                        pv = sp.tile([128, HPC * HS], F32, tag="pq", bufs=2,
                                     name="pv")
                        for cb in range(NCB):
                            nc.tensor.matmul(
                                pv[:], xT[:, cb, sb * 128:(sb + 1) * 128],
                                wv[:, cb, :],
                                start=(cb == 0), stop=(cb == NCB - 1))
                        for hh in range(HPC):
                            nc.vector.tensor_scalar_mul(
                                vN[:, sb, hh * (HS + 1):hh * (HS + 1) + HS],
                                pv[:, hh * HS:(hh + 1) * HS],
                                rstd_c[:, sb:sb + 1])

                # --- scores/softmax/AV ---
                with tc.tile_pool(name=f"sp2_{_rep}_{l}", bufs=1, space="PSUM") as sp:
                    oT = at.tile([128, 2, T], F32R, name=f"oT{l}")
                    for tt in range(NT):
                        n_sb = (tt + 1) * 4
                        tsl = slice(tt * 512, (tt + 1) * 512)
                        for w in range(2):
                            avp = [sp.tile([128, 512], F32, tag=f"av{i}",
                                           name=f"av{i}") for i in range(4)]
                            for sb in range(n_sb):
                                scp = sp.tile([128, 2048], F32, tag="sc",
                                              name="sc")
                                for q in range(4):
                                    nc.tensor.matmul(
                                        scp[:, q * 512:(q + 1) * 512],
                                        kT[32 * q:32 * q + 32, w,
                                           sb * 128:(sb + 1) * 128],
                                        qT[32 * q:32 * q + 32, w, tsl],
                                        start=True, stop=True,
                                        tile_position=(32 * q, 0))
                                P = at.tile([128, 2048], F32R, tag="P",
                                            bufs=1, name="P")
                                nc.scalar.activation(P[:], scp[:], AF.Exp)
                                m = sb - tt * 4
                                if m >= 0:
                                    nc.vector.tensor_mul(P[:], P[:],
                                                         masks[:, m, :])
                                first, last = (sb == 0), (sb == n_sb - 1)
                                for q in range(4):
                                    hl = q // 2
                                    hh = 2 * w + hl
                                    nc.tensor.matmul(
                                        avp[q][0:65, :],
                                        vN[:, sb, hh * (HS + 1):
                                           hh * (HS + 1) + HS + 1],
                                        P[:, q * 512:(q + 1) * 512],
                                        start=first, stop=last)
                            zr = at.tile([128, 512], F32R, tag="zr",
                                         name="zr")
                            bcs = []
                            for q in range(4):
                                nc.scalar.copy(zr[64:65, :],
                                               avp[q][64:65, :])
                                bcp = sp.tile([128, 512], F32, tag="sc",
                                              name=f"bc{q}")
                                nc.tensor.matmul(
                                    bcp[:], ones128[64:65, :],
                                    zr[64:65, :],
                                    start=True, stop=True,
                                    tile_position=(64, 0))
                                bc = at.tile([128, 512], F32,
                                             tag=f"bcs{q % 2}", bufs=1,
                                             name=f"bcsb{q}")
                                nc.vector.reciprocal_approx_fast(out=bc[:],
                                                                 in_=bcp[:])
                                bcs.append(bc)
                            scr = at.tile([128, 512], F32, tag="scr",
                                          name="scr")
                            oshift = at.tile([128, 512], F32R, tag="osh",
                                             bufs=2, name="oshift")
                            for hl in range(2):
                                h = 2 * w + hl
                                dst = (oT[0:64, w, tsl] if hl == 0
                                       else oshift[0:64, :])
                                nc.vector.tensor_mul(
                                    scr[0:64, :], avp[2 * hl + 1][0:64, :],
                                    bcs[2 * hl + 1][0:64, :])
                                nc.vector.tensor_mul(
                                    dst, avp[2 * hl][0:64, :],
                                    bcs[2 * hl][0:64, :])
                                nc.vector.scalar_tensor_tensor(
                                    out=dst, in0=scr[0:64, :],
                                    scalar=neglam[0:64, l, h:h + 1],
                                    in1=dst,
                                    op0=ALU.mult, op1=ALU.add)
                                if hl == 1:
                                    nc.sync.dma_start(
                                        out=oT[64:128, w, tsl],
                                        in_=oshift[0:64, :])

                # --- out_proj -> AR -> residual ---
                with tc.tile_pool(name=f"sp3_{_rep}_{l}", bufs=1, space="PSUM") as sp:
                    for tt in range(NT):
                        tsl = slice(tt * 512, (tt + 1) * 512)
                        for mb in range(NCB):
                            po = sp.tile([128, 512], F32, tag="pq", bufs=2,
                                         name="po")
                            for kb in range(2):
                                nc.tensor.matmul(
                                    po[:],
                                    ow[:, kb, mb * 128:(mb + 1) * 128],
                                    oT[:, kb, tsl],
                                    start=(kb == 0), stop=(kb == 1))
                            yst = at.tile([128, 512], F32, tag="yst", bufs=2,
                                          name="yst")
                            nc.scalar.copy(yst[:], po[:])
                            nc.sync.dma_start(
                                out=ar_in[:][mb * 128:(mb + 1) * 128, tsl],
                                in_=yst[:])
                    nc.gpsimd.collective_compute(
                        "AllReduce", ALU.add, replica_groups=GROUPS,
                        ins=[ar_in.opt()], outs=[ar_out.opt()])
                    for cb in range(NCB):
                        yf = at.tile([128, T], F32, tag="yf", bufs=2,
                                     name="yf")
                        nc.sync.dma_start(
                            out=yf[:],
                            in_=ar_out[:][cb * 128:(cb + 1) * 128, :])
                        nc.vector.tensor_add(xT[:, cb, :], xT[:, cb, :],
                                             yf[:])

              # ======== MLP ========
              with (
                  tc.tile_pool(name=f"ml{_rep}_{l}", bufs=1) as ml,
                  tc.tile_pool(name=f"mp{_rep}_{l}", bufs=1, space="PSUM") as mp,
              ):
                w1 = ml.tile([128, NCB, 2 * HIDS], F32R, name=f"w1{l}")
                w2 = ml.tile([128, HIDS // 128, C], F32R, name=f"w2{l}")
                nc.sync.dma_start(
                    out=w1[:],
                    in_=w1_e.ap()[l].rearrange("(a p) m -> p a m", p=128))
                nc.sync.dma_start(
                    out=w2[:],
                    in_=w2_e.ap()[l].rearrange("(a p) m -> p a m", p=128))
                ar_in2 = dram.tile([C, T], F32, tag="ari", name=f"ari_m{l}")
                ar_out2 = dram.tile([C, T], F32, tag="aro", name=f"aro_m{l}")

                sq2 = ml.tile([128, NCB, T], BF16, name=f"sq2{l}")
                for cb in range(NCB):
                    nc.scalar.activation(sq2[:, cb, :], xT[:, cb, :],
                                         AF.Square)
                ssq2 = rowstats(mp, sq2, "st2", ones_cb)
                std2 = ml.tile([1, T], F32R, name=f"std2{l}")
                nc.scalar.activation(std2[:], ssq2[:], AF.Sqrt,
                                     scale=1.0 / C, bias=EPS)
                rstd2b = bcast_row(mp, ml, std2, T, "r2", recip=True)

                NGB = HIDS // 128
                for tt in range(NT):
                    tsl = slice(tt * 512, (tt + 1) * 512)
                    gsb = ml.tile([128, NGB, 512], F32R, tag="gsb",
                                  name="gsb")
                    asb = ml.tile([128, NGB, 512], F32R, tag="asb",
                                  name="asb")
                    for gb in range(2 * NGB):
                        pu = mp.tile([128, 512], F32, tag=f"pu{gb % 4}",
                                     name=f"pu{gb}")
                        for cb in range(NCB):
                            nc.tensor.matmul(
                                pu[:], w1[:, cb, gb * 128:(gb + 1) * 128],
                                xT[:, cb, tsl],
                                start=(cb == 0), stop=(cb == NCB - 1))
                        if gb < NGB:
                            nc.vector.tensor_mul(gsb[:, gb, :], pu[:],
                                                 rstd2b[:, tsl])
                        else:
                            nc.vector.tensor_mul(asb[:, gb - NGB, :], pu[:],
                                                 rstd2b[:, tsl])
                    zsb = ml.tile([128, NGB, 512], F32R, tag="zsb",
                                  name="zsb")
                    for gb in range(NGB):
                        nc.scalar.activation(asb[:, gb, :], asb[:, gb, :],
                                             AF.Silu)
                        nc.vector.tensor_mul(zsb[:, gb, :], asb[:, gb, :],
                                             gsb[:, gb, :])
                    for mb in range(NCB):
                        pz = mp.tile([128, 512], F32, tag=f"pu{mb % 4}",
                                     name=f"pz{mb}")
                        for kb in range(NGB):
                            nc.tensor.matmul(
                                pz[:], w2[:, kb, mb * 128:(mb + 1) * 128],
                                zsb[:, kb, :],
                                start=(kb == 0), stop=(kb == NGB - 1))
                        zst = ml.tile([128, 512], F32, tag="zst", bufs=3,
                                      name="zst")
                        nc.scalar.copy(zst[:], pz[:])
                        nc.sync.dma_start(
                            out=ar_in2[:][mb * 128:(mb + 1) * 128, tsl],
                            in_=zst[:])
                nc.gpsimd.collective_compute(
                    "AllReduce", ALU.add, replica_groups=GROUPS,
                    ins=[ar_in2.opt()], outs=[ar_out2.opt()])
                for cb in range(NCB):
                    zf = ml.tile([128, T], F32, tag="zf", bufs=2, name="zf")
                    nc.sync.dma_start(
                        out=zf[:],
                        in_=ar_out2[:][cb * 128:(cb + 1) * 128, :])
                    nc.vector.tensor_add(xT[:, cb, :], xT[:, cb, :], zf[:])

            # ---------------- ln_f -> xc ----------------
            with (
                tc.tile_pool(name=f"hd0{_rep}", bufs=1) as hd0,
                tc.tile_pool(name=f"hps{_rep}", bufs=1, space="PSUM") as hp,
            ):
                mu_ps = rowstats(hp, xT, "mu", ones_col[:])
                sqf = hd0.tile([128, NCB, T], BF16, name="sqf")
                for cb in range(NCB):
                    nc.scalar.activation(sqf[:, cb, :], xT[:, cb, :],
                                         AF.Square)
                ms_ps = rowstats(hp, sqf, "ms", ones_cb)
                negmu = hd0.tile([1, T], F32R, name="negmu")
                nc.scalar.activation(negmu[:], mu_ps[:], AF.Copy,
                                     scale=-1.0 / C)
                mom2 = hd0.tile([1, T], F32, name="mom2")
                nc.scalar.activation(mom2[:], ms_ps[:], AF.Copy,
                                     scale=1.0 / C)
                mu2 = hd0.tile([1, T], F32, name="mu2")
                nc.vector.tensor_mul(mu2[:], negmu[:], negmu[:])
                var = hd0.tile([1, T], F32, name="var")
                nc.vector.tensor_sub(var[:], mom2[:], mu2[:])
                stdf = hd0.tile([1, T], F32R, name="stdf")
                nc.scalar.activation(stdf[:], var[:], AF.Sqrt, bias=EPS)
                negmu_b = bcast_row(hp, hd0, negmu, T, "nmu", recip=False)
                rstdf_b = bcast_row(hp, hd0, stdf, T, "rsf", recip=True)
                xc = pp.tile([128, NCB, T], F32R, name="xc")
                holder["xc"] = xc
                for cb in range(NCB):
                    nc.vector.tensor_add(xc[:, cb, :], xT[:, cb, :],
                                         negmu_b[:])
                    nc.vector.tensor_mul(xc[:, cb, :], xc[:, cb, :],
                                         rstdf_b[:])

          # ---------------- lm_head ----------------
          xc = holder["xc"]
          with (
              tc.tile_pool(name=f"hd{_rep}", bufs=1) as hd,
              tc.tile_pool(name=f"hw{_rep}", bufs=2) as hw,
              tc.tile_pool(name=f"hp2{_rep}", bufs=1, space="PSUM") as hp,
          ):
              NVT = VSH // 512            # 25
              VG = 2
              n_groups = (NVT + VG - 1) // VG
              for g in range(n_groups):
                  vts = list(range(g * VG, min((g + 1) * VG, NVT)))
                  nv = len(vts)
                  wt = hw.tile([128, NCB, VG * 512], F32R, tag="wt",
                               name=f"wt{g}")
                  nc.sync.dma_start(
                      out=wt[:, :, 0:nv * 512],
                      in_=wteT_e.ap()[:, vts[0] * 512:(vts[-1] + 1) * 512]
                          .rearrange("(a p) v -> p a v", p=128))
                  for tb in range(NSB):
                      lps = [hp.tile([128, 512], F32, tag=f"lg{i}",
                                     name=f"lg{g}_{tb}_{i}", bufs=2)
                             for i in range(nv)]
                      for cb in range(NCB):
                          for i in range(nv):
                              nc.tensor.matmul(
                                  lps[i][:],
                                  xc[:, cb, tb * 128:(tb + 1) * 128],
                                  wt[:, cb, i * 512:(i + 1) * 512],
                                  start=(cb == 0), stop=(cb == NCB - 1))
                      lo = hd.tile([128, VG * 512], F32, tag="lo", bufs=3,
                                   name=f"lo{g}_{tb}")
                      for i in range(nv):
                          if i % 2 == 0:
                              nc.scalar.copy(lo[:, i * 512:(i + 1) * 512],
                                             lps[i][:])
                          else:
                              nc.vector.tensor_copy(
                                  lo[:, i * 512:(i + 1) * 512], lps[i][:])
                      nc.sync.dma_start(
                          out=logits_e[tb * 128:(tb + 1) * 128,
                                       vts[0] * 512:(vts[-1] + 1) * 512],
                          in_=lo[:, 0:nv * 512])

    nc.compile()
    return nc


def _prep_inputs(inputs):
    import ml_dtypes
    idx = np.asarray(inputs["idx"]).astype(np.int64)
    wte = np.asarray(inputs["wte"], np.float32)
    wpe = np.asarray(inputs["wpe"], np.float32)
    rms1 = np.asarray(inputs["rms1_w"], np.float32)
    rms2 = np.asarray(inputs["rms2_w"], np.float32)
    wq = np.asarray(inputs["wq"], np.float32)
    wk = np.asarray(inputs["wk"], np.float32)
    wv = np.asarray(inputs["wv"], np.float32)
    lq1 = np.asarray(inputs["lq1"], np.float32)
    lq2 = np.asarray(inputs["lq2"], np.float32)
    lk1 = np.asarray(inputs["lk1"], np.float32)
    lk2 = np.asarray(inputs["lk2"], np.float32)
    out_w = np.asarray(inputs["out_w"], np.float32)
    out_b = np.asarray(inputs["out_b"], np.float32)
    mlp_w1 = np.asarray(inputs["mlp_w1"], np.float32)
    mlp_b1 = np.asarray(inputs["mlp_b1"], np.float32)
    cproj_w = np.asarray(inputs["cproj_w"], np.float32)
    cproj_b = np.asarray(inputs["cproj_b"], np.float32)
    lnf_w = np.asarray(inputs["lnf_w"], np.float32)

    assert not (np.any(out_b) or np.any(mlp_b1) or np.any(cproj_b)), \
        "nonzero biases not supported by this kernel build"

    depth = np.arange(L, dtype=np.float32)
    lam_init = 0.8 - 0.6 * np.exp(-0.3 * (depth - 1.0))
    lam = (np.exp((lq1 * lk1).sum(-1)) - np.exp((lq2 * lk2).sum(-1))
           + lam_init[:, None])

    wteE = wte[:BLK] + wpe
    scale = 1.0 / math.sqrt(D)
    wq_f = wq * rms1[:, :, None, None] * scale
    wk_f = wk * rms1[:, :, None, None]
    wv_f = wv * rms1[:, :, None, None]
    w1_f = mlp_w1 * rms2[:, :, None]
    wteT_full = np.ascontiguousarray((wte * lnf_w[None, :]).T)

    jj = np.arange(512)[None, :]
    ppp = np.arange(128)[:, None]
    masks = np.zeros((128, 4, 2048), np.float32)
    for m in range(4):
        one = (jj >= (ppp + 128 * m)).astype(np.float32)
        masks[:, m, :] = np.tile(one, (1, 4))
    masks = masks.astype(ml_dtypes.bfloat16)

    in_maps = []
    for c in range(N_CORES):
        b, r = c // TP, c % TP
        hsl = slice(r * HPC, (r + 1) * HPC)
        oneh = np.zeros((BLK, T), np.float32)
        oneh[idx[b], np.arange(T)] = 1.0
        g0 = r * HIDS
        a0 = 2 * C + r * HIDS
        w1_s = np.concatenate(
            [w1_f[:, :, g0:g0 + HIDS], w1_f[:, :, a0:a0 + HIDS]], axis=2)
        w2_s = cproj_w[:, g0:g0 + HIDS, :]
        wteT_s = np.zeros((C, VSH), np.float32)
        lo, hi = r * VSH, min((r + 1) * VSH, V)
        if hi > lo:
            wteT_s[:, 0:hi - lo] = wteT_full[:, lo:hi]
        in_maps.append({
            "oneh": oneh,
            "wteE": wteE,
            "wq": np.ascontiguousarray(
                wq_f[:, :, hsl].reshape(L, C, HPC * HS)),
            "wk": np.ascontiguousarray(
                wk_f[:, :, hsl].reshape(L, C, HPC * HS)),
            "wv": np.ascontiguousarray(
                wv_f[:, :, hsl].reshape(L, C, HPC * HS)),
            "ow": np.ascontiguousarray(
                out_w.reshape(L, H, HS, C)[:, hsl].reshape(L, HPC * HS, C)),
            "w1": np.ascontiguousarray(w1_s),
            "w2": np.ascontiguousarray(w2_s),
            "neglam": np.ascontiguousarray(-lam[:, hsl]),
            "masks": masks,
            "wteT": wteT_s,
        })
    return in_maps


def kernel(**inputs):
    from concourse.bass_utils import run_bass_kernel_spmd
    if "nc" not in _BUILT:
        _BUILT["nc"] = _build()
    nc = _BUILT["nc"]
    in_maps = _prep_inputs(inputs)
    res = run_bass_kernel_spmd(nc, in_maps, core_ids=list(range(N_CORES)))
    outs = []
    for b in range(B):
        parts = [res.results[b * TP + r]["logits"] for r in range(TP)]
        outs.append(np.concatenate(parts, axis=1)[:, :V])
    return np.stack(outs, axis=0).astype(np.float32)



# revision 14
# speedup vs baseline: 31.4874x; 31.4874x over previous
"""DifferentialGPT forward on 8 TRN2 NeuronCores (Bass/Tile).

DP2 over batch x TP4 (4 heads, 512 MLP pairs, 12800 vocab cols per core).
bf16 AllReduce (groups of 4) after out_proj and c_proj, split into T-halves
and software-pipelined against compute, including across the layer boundary
(layer l+1 half-0 stats/QKV/attention overlap layer l's last AllReduce).
Residual kept transposed xT [C(part), T(free)] in f32r; moving-side matmuls
f32r or bf16; lm_head bf16 with bf16 logits output (host casts to f32).
Bulk weight DMAs ride the Activation-engine HWDGE queue, AR staging and
output stores ride the SP queue.
"""
import math
import numpy as np

V, BLK, C, H, L = 50257, 1024, 1024, 16, 4
B, T = 2, 1024
HS = C // H          # 64
D = HS // 2          # 32
EPS = 1e-5
N_CORES = 8
TP = 4
HPC = H // TP        # 4
VSH = 12800
HIDS = 4 * C // TP // 2   # 512
NCB = C // 128       # 8
NT = T // 512        # 2
NSB = T // 128       # 8
NGB = HIDS // 128    # 4

_BUILT = {}


def _build(rep_count=1, no_collective=False):
    from concourse import bass, mybir, bacc
    import concourse.tile as tile

    F32 = mybir.dt.float32
    F32R = mybir.dt.float32r
    BF16 = mybir.dt.bfloat16
    AF = mybir.ActivationFunctionType
    ALU = mybir.AluOpType

    nc = bacc.Bacc("TRN2", target_bir_lowering=False, debug=False,
                   num_devices=N_CORES)
    for _cv in (EPS,):
        _ct = nc.alloc_sbuf_tensor(f"const-float32-{_cv}", [128, 1], F32)
        nc.gpsimd.memset(_ct.ap(), _cv)
        nc.const_aps.aps[(F32, _cv)] = _ct.ap()
    nc.all_engine_barrier()

    def EIN(name, shape, dt):
        return nc.dram_tensor(name, shape, dt, kind="ExternalInput")

    x0_e = EIN("x0", [C, T], F32R)
    wq_e = EIN("wq", [L, C, HPC * HS], F32R)
    wk_e = EIN("wk", [L, C, HPC * HS], F32R)
    wv_e = EIN("wv", [L, C, HPC * HS], F32R)
    ow_e = EIN("ow", [L, HPC * HS, C], BF16)
    w1_e = EIN("w1", [L, C, 2 * HIDS], F32R)
    w2_e = EIN("w2", [L, HIDS, C], BF16)
    neglam_e = EIN("neglam", [L, HPC], F32)
    masks_e = EIN("masks", [128, 4, 2048], BF16)
    wteT_e = EIN("wteT", [C, VSH], BF16)
    logits_e = nc.dram_tensor("logits", [T, VSH], BF16,
                              kind="ExternalOutput")

    GROUPS = [[0, 1, 2, 3], [4, 5, 6, 7]]

    with tile.TileContext(nc) as tc:
      with (
        tc.tile_pool(name="persist", bufs=1) as pp,
        tc.tile_pool(name="dram", bufs=1, space="DRAM") as dram,
      ):
        masks = pp.tile([128, 4, 1024], BF16, name="masks")
        nc.scalar.dma_start(out=masks[:], in_=masks_e[:, :, 0:1024])
        neglam = pp.tile([128, L, HPC], F32, name="neglam")
        nc.scalar.dma_start(
            out=neglam[:],
            in_=bass.AP(tensor=neglam_e, offset=0,
                        ap=[[0, 128], [HPC, L], [1, HPC]]))
        ones_f = pp.tile([128, 128], F32, name="ones_f")
        nc.vector.memset(ones_f[:], 1.0)
        ones128 = pp.tile([128, 128], F32R, name="ones128")
        nc.vector.tensor_copy(ones128[:], ones_f[:])
        ones128b = pp.tile([128, 128], BF16, name="ones128b")
        nc.vector.tensor_copy(ones128b[:], ones_f[:])
        ones_cb = ones128b[:, 0:1]

        for _rep in range(rep_count):
          xpool = tc.alloc_tile_pool(name=f"xpool{_rep}", bufs=1)
          xT = xpool.tile([128, NCB, T], F32R, name="xT")
          for tt in range(NT):
              tsl = slice(tt * 512, (tt + 1) * 512)
              nc.scalar.dma_start(
                  out=xT[:, :, tsl],
                  in_=x0_e.ap()[:, tsl].rearrange("(a p) t -> p a t",
                                                  p=128))

          with tc.tile_pool(name=f"ps{_rep}", bufs=1, space="PSUM") as sp:

            def psum_big(name):
                return sp.tile([128, 1024], F32, tag="sc", bufs=2,
                               name=name)

            # rmsnorm stats for half tt: rstd broadcast [128,512] and
            # per-token rstd columns [128,4] (token in partition, by sb)
            def stats_half(pool, tt, tag, want_cols=True):
                tsl = slice(tt * 512, (tt + 1) * 512)
                sq = xpool.tile([128, NCB, 512], BF16, tag="sq", bufs=1,
                                name=f"sq{tag}{tt}")
                for cb in range(NCB):
                    nc.scalar.activation(sq[:, cb, :], xT[:, cb, tsl],
                                         AF.Square)
                big = psum_big(f"ssq{tag}{tt}")
                ssq_b = big[:, 0:512]
                for cb in range(NCB):
                    nc.tensor.matmul(ssq_b, ones128b[:], sq[:, cb, :],
                                     start=(cb == 0),
                                     stop=(cb == NCB - 1))
                if want_cols:
                    for sb4 in range(4):
                        for cb in range(NCB):
                            nc.tensor.matmul(
                                big[:, 512 + sb4:513 + sb4],
                                sq[:, cb, sb4 * 128:(sb4 + 1) * 128],
                                ones_cb,
                                start=(cb == 0), stop=(cb == NCB - 1))
                std_b = pool.tile([128, 512], F32, tag=f"sdb{tag}",
                                  name=f"sdb{tag}{tt}")
                nc.scalar.activation(std_b[:], ssq_b, AF.Sqrt,
                                     scale=1.0 / C, bias=EPS)
                rstd_b = pool.tile([128, 512], F32, tag=f"rsb{tag}",
                                   name=f"rsb{tag}{tt}")
                nc.vector.reciprocal_approx_fast(out=rstd_b[:],
                                                 in_=std_b[:])
                rstd_c = None
                if want_cols:
                    std_c = pool.tile([128, 8], F32, tag=f"sdc{tag}",
                                      name=f"sdc{tag}{tt}")
                    nc.scalar.activation(std_c[:, 0:4],
                                         big[:, 512:516], AF.Sqrt,
                                         scale=1.0 / C, bias=EPS)
                    rstd_c = pool.tile([128, 8], F32, tag=f"rsc{tag}",
                                       name=f"rsc{tag}{tt}")
                    nc.vector.reciprocal_approx_fast(
                        out=rstd_c[:, 0:4], in_=std_c[:, 0:4])
                return rstd_b, rstd_c

            pend_h1_final = [None]

            for l in range(L):
                at = tc.alloc_tile_pool(name=f"at{_rep}_{l}", bufs=1)
                wq = at.tile([128, NCB, HPC * HS], F32R, name=f"wq{l}")
                wk = at.tile([128, NCB, HPC * HS], F32R, name=f"wk{l}")
                wv = at.tile([128, NCB, HPC * HS], F32R, name=f"wv{l}")
                ow = at.tile([128, (HPC * HS) // 128, C], BF16,
                             name=f"ow{l}")
                for t_sb, t_e in ((wq, wq_e), (wk, wk_e), (wv, wv_e),
                                  (ow, ow_e)):
                    nc.scalar.dma_start(
                        out=t_sb[:],
                        in_=t_e.ap()[l].rearrange("(a p) m -> p a m",
                                                  p=128))

                qT = at.tile([128, 2, T], F32R, name=f"qT{l}")
                kT = at.tile([128, 2, T], F32R, name=f"kT{l}")
                vN = at.tile([128, NSB, HPC * (HS + 1)], BF16,
                             name=f"vN{l}")
                vN4 = vN[:].rearrange("p s (h e) -> p s h e", e=HS + 1)
                oT = at.tile([128, 2, T], BF16, name=f"oT{l}")

                def qkv_half(tt, rstd_b, rstd_c):
                    tsl = slice(tt * 512, (tt + 1) * 512)
                    for dst, w_sb in ((qT, wq), (kT, wk)):
                        for mb in range(2):
                            pq = psum_big("pq")[:, 0:512]
                            for cb in range(NCB):
                                nc.tensor.matmul(
                                    pq,
                                    w_sb[:, cb, mb * 128:(mb + 1) * 128],
                                    xT[:, cb, tsl],
                                    start=(cb == 0),
                                    stop=(cb == NCB - 1))
                            nc.vector.tensor_mul(dst[:, mb, tsl], pq,
                                                 rstd_b[:])
                    for sb4 in range(4):
                        sb = tt * 4 + sb4
                        nc.vector.tensor_copy(vN4[:, sb, :, HS:HS + 1],
                                              ones_f[:, 0:HPC])
                        pv = psum_big("pv")[:, 0:512]
                        for cb in range(NCB):
                            nc.tensor.matmul(
                                pv[:, 0:HPC * HS],
                                xT[:, cb, sb * 128:(sb + 1) * 128],
                                wv[:, cb, :],
                                start=(cb == 0), stop=(cb == NCB - 1))
                        for hh in range(HPC):
                            nc.vector.tensor_scalar_mul(
                                vN[:, sb,
                                   hh * (HS + 1):hh * (HS + 1) + HS],
                                pv[:, hh * HS:(hh + 1) * HS],
                                rstd_c[:, sb4:sb4 + 1])

                def attn_tt(tt):
                    n_sb = (tt + 1) * 4
                    tsl = slice(tt * 512, (tt + 1) * 512)
                    for w in range(2):
                        avp = [sp.tile([128, 512], F32, tag=f"av{i}",
                                       name=f"av{i}") for i in range(4)]
                        pend_av = []

                        def do_av(sb, Ps):
                            first, last = (sb == 0), (sb == n_sb - 1)
                            for hp in range(2):
                                hh = 2 * w + hp
                                for qq in range(2):
                                    q = 2 * hp + qq
                                    nc.tensor.matmul(
                                        avp[q][0:65, :],
                                        vN[:, sb, hh * (HS + 1):
                                           hh * (HS + 1) + HS + 1],
                                        Ps[hp][:,
                                               qq * 512:(qq + 1) * 512],
                                        start=first, stop=last)

                        for sb in range(n_sb):
                            Ps = []
                            for hp in range(2):
                                scp = psum_big(f"sc{hp}")
                                for qq in range(2):
                                    q = 2 * hp + qq
                                    nc.tensor.matmul(
                                        scp[:, qq * 512:(qq + 1) * 512],
                                        kT[32 * q:32 * q + 32, w,
                                           sb * 128:(sb + 1) * 128],
                                        qT[32 * q:32 * q + 32, w, tsl],
                                        start=True, stop=True,
                                        tile_position=(32 * q, 0))
                                P = at.tile([128, 1024], BF16,
                                            tag=f"P{hp}", bufs=2,
                                            name=f"P{hp}")
                                nc.scalar.activation(P[:], scp[:],
                                                     AF.Exp)
                                m = sb - tt * 4
                                if m >= 0:
                                    nc.vector.tensor_mul(
                                        P[:], P[:], masks[:, m, :])
                                Ps.append(P)
                            pend_av.append((sb, Ps))
                            if len(pend_av) > 1:
                                do_av(*pend_av.pop(0))
                        do_av(*pend_av.pop(0))

                        # combine dual softmax -> oT
                        zr = at.tile([128, 512], F32R, tag="zr",
                                     name="zr")
                        bcs = []
                        for q in range(4):
                            nc.scalar.copy(zr[64:65, :],
                                           avp[q][64:65, :])
                            bcp = psum_big(f"bc{q}")[:, 0:512]
                            nc.tensor.matmul(
                                bcp, ones128[64:65, :], zr[64:65, :],
                                start=True, stop=True,
                                tile_position=(64, 0))
                            bc = at.tile([128, 512], F32,
                                         tag=f"bcs{q % 2}", bufs=1,
                                         name=f"bcsb{q}")
                            nc.vector.reciprocal_approx_fast(
                                out=bc[:], in_=bcp)
                            bcs.append(bc)
                        scr = at.tile([128, 512], F32, tag="scr",
                                      name="scr")
                        oshift = at.tile([128, 512], BF16, tag="osh",
                                         bufs=2, name="oshift")
                        for hl in range(2):
                            h = 2 * w + hl
                            dst = (oT[0:64, w, tsl] if hl == 0
                                   else oshift[0:64, :])
                            nc.vector.tensor_mul(
                                scr[0:64, :], avp[2 * hl + 1][0:64, :],
                                bcs[2 * hl + 1][0:64, :])
                            nc.vector.tensor_mul(
                                dst, avp[2 * hl][0:64, :],
                                bcs[2 * hl][0:64, :])
                            nc.vector.scalar_tensor_tensor(
                                out=dst, in0=scr[0:64, :],
                                scalar=neglam[0:64, l, h:h + 1],
                                in1=dst,
                                op0=ALU.mult, op1=ALU.add)
                            if hl == 1:
                                nc.sync.dma_start(
                                    out=oT[64:128, w, tsl],
                                    in_=oshift[0:64, :])

                arA_in = [dram.tile([C, 512], BF16, tag=f"ariA{tt}",
                                    name=f"ariA{tt}_{l}")
                          for tt in range(2)]
                arA_out = [dram.tile([C, 512], BF16, tag=f"aroA{tt}",
                                     name=f"aroA{tt}_{l}")
                           for tt in range(2)]

                def op_half(tt):
                    tsl = slice(tt * 512, (tt + 1) * 512)
                    for mb in range(NCB):
                        po = psum_big("po")[:, 0:512]
                        for kb in range(2):
                            nc.tensor.matmul(
                                po,
                                ow[:, kb, mb * 128:(mb + 1) * 128],
                                oT[:, kb, tsl],
                                start=(kb == 0), stop=(kb == 1))
                        yst = at.tile([128, 512], BF16, tag="yst",
                                      bufs=2, name="yst")
                        nc.scalar.copy(yst[:], po)
                        nc.sync.dma_start(
                            out=arA_in[tt][:][mb * 128:(mb + 1) * 128,
                                              :],
                            in_=yst[:])
                    if no_collective:
                        nc.sync.dma_start(out=arA_out[tt][:],
                                          in_=arA_in[tt][:])
                    else:
                        nc.gpsimd.collective_compute(
                            "AllReduce", ALU.add, replica_groups=GROUPS,
                            ins=[arA_in[tt].opt()],
                            outs=[arA_out[tt].opt()])

                # ---- emission schedule ----
                rstd_b0, rstd_c0 = stats_half(at, 0, "a")
                qkv_half(0, rstd_b0, rstd_c0)
                attn_tt(0)
                op_half(0)                      # -> AR(0)
                if pend_h1_final[0] is not None:
                    pend_h1_final[0]()          # resid M(l-1, h1)
                    pend_h1_final[0] = None
                rstd_b1, rstd_c1 = stats_half(at, 1, "a")
                qkv_half(1, rstd_b1, rstd_c1)
                attn_tt(1)                      # overlaps AR(0)
                op_half(1)                      # -> AR(1)
                at.release()

                # ======== MLP ========
                ml = tc.alloc_tile_pool(name=f"ml{_rep}_{l}", bufs=1, side="right")
                w1 = ml.tile([128, NCB, 2 * HIDS], F32R, name=f"w1{l}")
                w2 = ml.tile([128, NGB, C], BF16, name=f"w2{l}")
                nc.scalar.dma_start(
                    out=w1[:],
                    in_=w1_e.ap()[l].rearrange("(a p) m -> p a m",
                                               p=128))
                nc.scalar.dma_start(
                    out=w2[:],
                    in_=w2_e.ap()[l].rearrange("(a p) m -> p a m",
                                               p=128))
                arM_in = [dram.tile([C, 512], BF16, tag=f"ariM{tt}",
                                    name=f"ariM{tt}_{l}")
                          for tt in range(2)]
                arM_out = [dram.tile([C, 512], BF16, tag=f"aroM{tt}",
                                     name=f"aroM{tt}_{l}")
                           for tt in range(2)]

                def resid_half(tt, ar_out, tag):
                    tsl = slice(tt * 512, (tt + 1) * 512)
                    for cb in range(NCB):
                        yf = ml.tile([128, 512], BF16, tag=f"yf{tag}",
                                     bufs=2, name=f"yf{tag}")
                        nc.sync.dma_start(
                            out=yf[:],
                            in_=ar_out[:][cb * 128:(cb + 1) * 128, :])
                        nc.vector.tensor_add(xT[:, cb, tsl],
                                             xT[:, cb, tsl], yf[:])

                def mlp_half(tt, rstd2b, gsb, asb):
                    tsl = slice(tt * 512, (tt + 1) * 512)
                    for gb in range(2 * NGB):
                        pu = psum_big(f"pu{gb}")[:, 0:512]
                        for cb in range(NCB):
                            nc.tensor.matmul(
                                pu,
                                w1[:, cb, gb * 128:(gb + 1) * 128],
                                xT[:, cb, tsl],
                                start=(cb == 0), stop=(cb == NCB - 1))
                        if gb < NGB:
                            nc.vector.tensor_mul(gsb[:, gb, :], pu,
                                                 rstd2b[:])
                        else:
                            nc.vector.tensor_mul(asb[:, gb - NGB, :],
                                                 pu, rstd2b[:])
                    for gb in range(NGB):
                        nc.scalar.activation(asb[:, gb, :],
                                             asb[:, gb, :], AF.Silu)
                        nc.vector.tensor_mul(gsb[:, gb, :],
                                             asb[:, gb, :],
                                             gsb[:, gb, :])
                    for mb in range(NCB):
                        pz = psum_big(f"pz{mb}")[:, 0:512]
                        for kb in range(NGB):
                            nc.tensor.matmul(
                                pz,
                                w2[:, kb, mb * 128:(mb + 1) * 128],
                                gsb[:, kb, :],
                                start=(kb == 0), stop=(kb == NGB - 1))
                        zst = ml.tile([128, 512], BF16, tag="zst",
                                      bufs=2, name="zst")
                        nc.scalar.copy(zst[:], pz)
                        nc.sync.dma_start(
                            out=arM_in[tt][:][mb * 128:(mb + 1) * 128,
                                              :],
                            in_=zst[:])
                    if no_collective:
                        nc.sync.dma_start(out=arM_out[tt][:],
                                          in_=arM_in[tt][:])
                    else:
                        nc.gpsimd.collective_compute(
                            "AllReduce", ALU.add, replica_groups=GROUPS,
                            ins=[arM_in[tt].opt()],
                            outs=[arM_out[tt].opt()])

                gsb = [ml.tile([128, NGB, 512], BF16, name=f"gsb{i}")
                       for i in range(2)]
                asb = [ml.tile([128, NGB, 512], BF16, name=f"asb{i}")
                       for i in range(2)]

                resid_half(0, arA_out[0], "a")
                rstd2b0, _ = stats_half(ml, 0, "m", want_cols=False)
                mlp_half(0, rstd2b0, gsb[0], asb[0])    # -> AR2(0)
                resid_half(1, arA_out[1], "a")
                rstd2b1, _ = stats_half(ml, 1, "m", want_cols=False)
                mlp_half(1, rstd2b1, gsb[1], asb[1])    # -> AR2(1)
                resid_half(0, arM_out[0], "m")

                def make_final(ml_pool_, arM_out1_):
                    def fin():
                        tsl = slice(512, 1024)
                        for cb in range(NCB):
                            zf = ml_pool_.tile([128, 512], BF16,
                                               tag="yfm", bufs=2,
                                               name="zf")
                            nc.sync.dma_start(
                                out=zf[:],
                                in_=arM_out1_[:][cb * 128:
                                                 (cb + 1) * 128, :])
                            nc.vector.tensor_add(xT[:, cb, tsl],
                                                 xT[:, cb, tsl],
                                                 zf[:])
                        ml_pool_.release()
                    return fin

                pend_h1_final[0] = make_final(ml, arM_out[1])

            if pend_h1_final[0] is not None:
                pend_h1_final[0]()
                pend_h1_final[0] = None

            # ---------------- ln_f -> xc (bf16) ----------------
            hd = tc.alloc_tile_pool(name=f"hd{_rep}", bufs=1)
            xc = hd.tile([128, NCB, T], BF16, name="xc")
            hd0 = tc.alloc_tile_pool(name=f"hd0{_rep}", bufs=1)
            for tt in range(NT):
                tsl = slice(tt * 512, (tt + 1) * 512)
                big = psum_big("mu_ms")
                mu_b = big[:, 0:512]
                for cb in range(NCB):
                    nc.tensor.matmul(mu_b, ones128[:], xT[:, cb, tsl],
                                     start=(cb == 0),
                                     stop=(cb == NCB - 1))
                sqf = xpool.tile([128, NCB, 512], BF16, tag="sq",
                                 bufs=1, name="sqf")
                for cb in range(NCB):
                    nc.scalar.activation(sqf[:, cb, :], xT[:, cb, tsl],
                                         AF.Square)
                ms_b = big[:, 512:1024]
                for cb in range(NCB):
                    nc.tensor.matmul(ms_b, ones128b[:], sqf[:, cb, :],
                                     start=(cb == 0),
                                     stop=(cb == NCB - 1))
                negmu = hd0.tile([128, 512], F32, tag="negmu",
                                 name="negmu")
                nc.scalar.activation(negmu[:], mu_b, AF.Copy,
                                     scale=-1.0 / C)
                mom2 = hd0.tile([128, 512], F32, tag="mom2",
                                name="mom2")
                nc.scalar.activation(mom2[:], ms_b, AF.Copy,
                                     scale=1.0 / C)
                mu2 = hd0.tile([128, 512], F32, tag="mu2", name="mu2")
                nc.vector.tensor_mul(mu2[:], negmu[:], negmu[:])
                var = hd0.tile([128, 512], F32, tag="var", name="var")
                nc.vector.tensor_sub(var[:], mom2[:], mu2[:])
                stdf = hd0.tile([128, 512], F32, tag="stdf",
                                name="stdf")
                nc.scalar.activation(stdf[:], var[:], AF.Sqrt, bias=EPS)
                rstdf = hd0.tile([128, 512], F32, tag="rstdf",
                                 name="rstdf")
                nc.vector.reciprocal_approx_fast(out=rstdf[:],
                                                 in_=stdf[:])
                for cb in range(NCB):
                    xcen = hd0.tile([128, 512], F32, tag="xcen",
                                    bufs=2, name="xcen")
                    nc.vector.tensor_add(xcen[:], xT[:, cb, tsl],
                                         negmu[:])
                    nc.vector.tensor_mul(xc[:, cb, tsl], xcen[:],
                                         rstdf[:])
            hd0.release()

            # ---------------- lm_head (bf16) ----------------
            with tc.tile_pool(name=f"hw{_rep}", bufs=2) as hw:
                NVT = VSH // 512            # 25
                VG = 2
                n_groups = (NVT + VG - 1) // VG
                for g in range(n_groups):
                    vts = list(range(g * VG, min((g + 1) * VG, NVT)))
                    nv = len(vts)
                    wt = hw.tile([128, NCB, VG * 512], BF16, tag="wt",
                                 name=f"wt{g}")
                    nc.scalar.dma_start(
                        out=wt[:, :, 0:nv * 512],
                        in_=wteT_e.ap()[:, vts[0] * 512:
                                        (vts[-1] + 1) * 512]
                            .rearrange("(a p) v -> p a v", p=128))
                    for tb in range(NSB):
                        po = 2 * (tb % 2)
                        lps = [sp.tile([128, 512], F32,
                                       tag=f"av{po + i}",
                                       name=f"lg{g}_{tb}_{i}")
                               for i in range(nv)]
                        for cb in range(NCB):
                            for i in range(nv):
                                nc.tensor.matmul(
                                    lps[i][:],
                                    xc[:, cb,
                                       tb * 128:(tb + 1) * 128],
                                    wt[:, cb, i * 512:(i + 1) * 512],
                                    start=(cb == 0),
                                    stop=(cb == NCB - 1))
                        lo = hd.tile([128, VG * 512], BF16, tag="lo",
                                     bufs=3, name=f"lo{g}_{tb}")
                        for i in range(nv):
                            if i % 2 == 0:
                                nc.scalar.copy(
                                    lo[:, i * 512:(i + 1) * 512],
                                    lps[i][:])
                            else:
                                nc.vector.tensor_copy(
                                    lo[:, i * 512:(i + 1) * 512],
                                    lps[i][:])
                        nc.sync.dma_start(
                            out=logits_e[tb * 128:(tb + 1) * 128,
                                         vts[0] * 512:
                                         (vts[-1] + 1) * 512],
                            in_=lo[:, 0:nv * 512])
            hd.release()

          xpool.release()

    nc.compile()
    return nc


def _prep_inputs(inputs):
    import ml_dtypes
    idx = np.asarray(inputs["idx"]).astype(np.int64)
    wte = np.asarray(inputs["wte"], np.float32)
    wpe = np.asarray(inputs["wpe"], np.float32)
    rms1 = np.asarray(inputs["rms1_w"], np.float32)
    rms2 = np.asarray(inputs["rms2_w"], np.float32)
    wq = np.asarray(inputs["wq"], np.float32)
    wk = np.asarray(inputs["wk"], np.float32)
    wv = np.asarray(inputs["wv"], np.float32)
    lq1 = np.asarray(inputs["lq1"], np.float32)
    lq2 = np.asarray(inputs["lq2"], np.float32)
    lk1 = np.asarray(inputs["lk1"], np.float32)
    lk2 = np.asarray(inputs["lk2"], np.float32)
    out_w = np.asarray(inputs["out_w"], np.float32)
    out_b = np.asarray(inputs["out_b"], np.float32)
    mlp_w1 = np.asarray(inputs["mlp_w1"], np.float32)
    mlp_b1 = np.asarray(inputs["mlp_b1"], np.float32)
    cproj_w = np.asarray(inputs["cproj_w"], np.float32)
    cproj_b = np.asarray(inputs["cproj_b"], np.float32)
    lnf_w = np.asarray(inputs["lnf_w"], np.float32)

    assert not (np.any(out_b) or np.any(mlp_b1) or np.any(cproj_b)), \
        "nonzero biases not supported by this kernel build"

    depth = np.arange(L, dtype=np.float32)
    lam_init = 0.8 - 0.6 * np.exp(-0.3 * (depth - 1.0))
    lam = (np.exp((lq1 * lk1).sum(-1)) - np.exp((lq2 * lk2).sum(-1))
           + lam_init[:, None])

    wteE = wte[:BLK] + wpe
    scale = 1.0 / math.sqrt(D)
    wq_f = wq * rms1[:, :, None, None] * scale
    wk_f = wk * rms1[:, :, None, None]
    wv_f = wv * rms1[:, :, None, None]
    w1_f = mlp_w1 * rms2[:, :, None]
    wteT_full = np.ascontiguousarray((wte * lnf_w[None, :]).T)

    jj = np.arange(512)[None, :]
    ppp = np.arange(128)[:, None]
    masks = np.zeros((128, 4, 2048), np.float32)
    for m in range(4):
        one = (jj >= (ppp + 128 * m)).astype(np.float32)
        masks[:, m, :] = np.tile(one, (1, 4))
    masks = masks.astype(ml_dtypes.bfloat16)

    in_maps = []
    for c in range(N_CORES):
        b, r = c // TP, c % TP
        hsl = slice(r * HPC, (r + 1) * HPC)
        x0 = np.ascontiguousarray(wteE[idx[b]].T)
        g0 = r * HIDS
        a0 = 2 * C + r * HIDS
        w1_s = np.concatenate(
            [w1_f[:, :, g0:g0 + HIDS], w1_f[:, :, a0:a0 + HIDS]],
            axis=2)
        w2_s = cproj_w[:, g0:g0 + HIDS, :]
        wteT_s = np.zeros((C, VSH), np.float32)
        lo, hi = r * VSH, min((r + 1) * VSH, V)
        if hi > lo:
            wteT_s[:, 0:hi - lo] = wteT_full[:, lo:hi]
        in_maps.append({
            "x0": x0,
            "wq": np.ascontiguousarray(
                wq_f[:, :, hsl].reshape(L, C, HPC * HS)),
            "wk": np.ascontiguousarray(
                wk_f[:, :, hsl].reshape(L, C, HPC * HS)),
            "wv": np.ascontiguousarray(
                wv_f[:, :, hsl].reshape(L, C, HPC * HS)),
            "ow": np.ascontiguousarray(
                out_w.reshape(L, H, HS, C)[:, hsl].reshape(
                    L, HPC * HS, C)).astype(ml_dtypes.bfloat16),
            "w1": np.ascontiguousarray(w1_s),
            "w2": np.ascontiguousarray(w2_s).astype(ml_dtypes.bfloat16),
            "neglam": np.ascontiguousarray(-lam[:, hsl]),
            "masks": masks,
            "wteT": wteT_s.astype(ml_dtypes.bfloat16),
        })
    return in_maps


def kernel(**inputs):
    from concourse.bass_utils import run_bass_kernel_spmd
    if "nc" not in _BUILT:
        _BUILT["nc"] = _build()
    nc = _BUILT["nc"]
    in_maps = _prep_inputs(inputs)
    res = run_bass_kernel_spmd(nc, in_maps, core_ids=list(range(N_CORES)))
    outs = []
    for b in range(B):
        parts = [res.results[b * TP + r]["logits"].astype(np.float32)
                 for r in range(TP)]
        outs.append(np.concatenate(parts, axis=1)[:, :V])
    return np.stack(outs, axis=0)


# revision 23
# speedup vs baseline: 46.4276x; 1.4745x over previous
"""DifferentialGPT forward on 8 TRN2 NeuronCores (Bass/Tile).

DP2 over batch x TP4 (4 heads, 512 MLP pairs, 12800 vocab cols per core).
bf16 AllReduce (groups of 4) after out_proj and c_proj, split into T-halves
and software-pipelined against compute, including across the layer boundary
(layer l+1 half-0 stats/QKV/attention overlap layer l's last AllReduce).
Residual kept transposed xT [C(part), T(free)] in f32r; moving-side matmuls
f32r or bf16; lm_head bf16 with bf16 logits output (host casts to f32).
Bulk weight DMAs ride the Activation-engine HWDGE queue, AR staging and
output stores ride the SP queue.
"""
import math
import numpy as np

V, BLK, C, H, L = 50257, 1024, 1024, 16, 4
B, T = 2, 1024
HS = C // H          # 64
D = HS // 2          # 32
EPS = 1e-5
N_CORES = 8
TP = 4
HPC = H // TP        # 4
VSH = 12800
HIDS = 4 * C // TP // 2   # 512
NCB = C // 128       # 8
NT = T // 512        # 2
NSB = T // 128       # 8
NGB = HIDS // 128    # 4

_BUILT = {}


def _build(rep_count=1, no_collective=False, ar_mode="half"):
    # ar_mode: "half"  - one AllReduce per T-half (default)
    #          "full"  - one AllReduce per sublayer over full T
    #          "nowait"- half ARs issued, but residual reads the un-reduced
    #                    input (WRONG results; timing diagnostic only)
    from concourse import bass, mybir, bacc
    import concourse.tile as tile

    F32 = mybir.dt.float32
    F32R = mybir.dt.float32r
    BF16 = mybir.dt.bfloat16
    AF = mybir.ActivationFunctionType
    ALU = mybir.AluOpType

    nc = bacc.Bacc("TRN2", target_bir_lowering=False, debug=False,
                   num_devices=N_CORES)
    for _cv in (EPS,):
        _ct = nc.alloc_sbuf_tensor(f"const-float32-{_cv}", [128, 1], F32)
        nc.gpsimd.memset(_ct.ap(), _cv)
        nc.const_aps.aps[(F32, _cv)] = _ct.ap()
    nc.all_engine_barrier()

    def EIN(name, shape, dt):
        return nc.dram_tensor(name, shape, dt, kind="ExternalInput")

    x0_e = EIN("x0", [C, T], F32R)
    wq_e = EIN("wq", [L, C, HPC * HS], F32R)
    wk_e = EIN("wk", [L, C, HPC * HS], F32R)
    wv_e = EIN("wv", [L, C, HPC * HS], F32R)
    ow_e = EIN("ow", [L, HPC * HS, C], BF16)
    w1_e = EIN("w1", [L, C, 2 * HIDS], F32R)
    w2_e = EIN("w2", [L, HIDS, C], BF16)
    neglam_e = EIN("neglam", [L, HPC], F32)
    masks_e = EIN("masks", [128, 4, 2048], BF16)
    wteT_e = EIN("wteT", [C, VSH], BF16)
    logits_e = nc.dram_tensor("logits", [T, VSH], BF16,
                              kind="ExternalOutput")

    GROUPS = [[0, 1, 2, 3], [4, 5, 6, 7]]

    with tile.TileContext(nc) as tc:
      with (
        tc.tile_pool(name="persist", bufs=1) as pp,
        tc.tile_pool(name="dram", bufs=1, space="DRAM") as dram,
      ):
        masks = pp.tile([128, 4, 1024], BF16, name="masks")
        nc.scalar.dma_start(out=masks[:], in_=masks_e[:, :, 0:1024])
        neglam = pp.tile([128, L, HPC], F32, name="neglam")
        nc.scalar.dma_start(
            out=neglam[:],
            in_=bass.AP(tensor=neglam_e, offset=0,
                        ap=[[0, 128], [HPC, L], [1, HPC]]))
        ones_f = pp.tile([128, 128], F32, name="ones_f")
        nc.vector.memset(ones_f[:], 1.0)
        ones128 = pp.tile([128, 128], F32R, name="ones128")
        nc.vector.tensor_copy(ones128[:], ones_f[:])
        ones128b = pp.tile([128, 128], BF16, name="ones128b")
        nc.vector.tensor_copy(ones128b[:], ones_f[:])
        ones_cb = ones128b[:, 0:1]

        for _rep in range(rep_count):
          xpool = tc.alloc_tile_pool(name=f"xpool{_rep}", bufs=1)
          xT = xpool.tile([128, NCB, T], F32R, name="xT")
          for tt in range(NT):
              tsl = slice(tt * 512, (tt + 1) * 512)
              nc.scalar.dma_start(
                  out=xT[:, :, tsl],
                  in_=x0_e.ap()[:, tsl].rearrange("(a p) t -> p a t",
                                                  p=128))

          with tc.tile_pool(name=f"ps{_rep}", bufs=1, space="PSUM") as sp:

            def psum_big(name):
                return sp.tile([128, 1024], F32, tag="sc", bufs=2,
                               name=name)

            # rmsnorm stats for half tt: rstd broadcast [128,512] and
            # per-token rstd columns [128,4] (token in partition, by sb)
            def stats_half(pool, tt, tag, want_cols=True):
                tsl = slice(tt * 512, (tt + 1) * 512)
                sq = xpool.tile([128, NCB, 512], BF16, tag="sq", bufs=1,
                                name=f"sq{tag}{tt}")
                for cb in range(NCB):
                    nc.scalar.activation(sq[:, cb, :], xT[:, cb, tsl],
                                         AF.Square)
                big = psum_big(f"ssq{tag}{tt}")
                ssq_b = big[:, 0:512]
                for cb in range(NCB):
                    nc.tensor.matmul(ssq_b, ones128b[:], sq[:, cb, :],
                                     start=(cb == 0),
                                     stop=(cb == NCB - 1))
                if want_cols:
                    for sb4 in range(4):
                        for cb in range(NCB):
                            nc.tensor.matmul(
                                big[:, 512 + sb4:513 + sb4],
                                sq[:, cb, sb4 * 128:(sb4 + 1) * 128],
                                ones_cb,
                                start=(cb == 0), stop=(cb == NCB - 1))
                std_b = pool.tile([128, 512], F32, tag=f"sdb{tag}",
                                  name=f"sdb{tag}{tt}")
                nc.scalar.activation(std_b[:], ssq_b, AF.Sqrt,
                                     scale=1.0 / C, bias=EPS)
                rstd_b = pool.tile([128, 512], F32, tag=f"rsb{tag}",
                                   name=f"rsb{tag}{tt}")
                nc.vector.reciprocal_approx_fast(out=rstd_b[:],
                                                 in_=std_b[:])
                rstd_c = None
                if want_cols:
                    std_c = pool.tile([128, 8], F32, tag=f"sdc{tag}",
                                      name=f"sdc{tag}{tt}")
                    nc.scalar.activation(std_c[:, 0:4],
                                         big[:, 512:516], AF.Sqrt,
                                         scale=1.0 / C, bias=EPS)
                    rstd_c = pool.tile([128, 8], F32, tag=f"rsc{tag}",
                                       name=f"rsc{tag}{tt}")
                    nc.vector.reciprocal_approx_fast(
                        out=rstd_c[:, 0:4], in_=std_c[:, 0:4])
                return rstd_b, rstd_c

            pend_h1_final = [None]

            for l in range(L):
                at = tc.alloc_tile_pool(name=f"at{_rep}_{l}", bufs=1)
                wq = at.tile([128, NCB, HPC * HS], F32R, name=f"wq{l}")
                wk = at.tile([128, NCB, HPC * HS], F32R, name=f"wk{l}")
                wv = at.tile([128, NCB, HPC * HS], F32R, name=f"wv{l}")
                ow = at.tile([128, (HPC * HS) // 128, C], BF16,
                             name=f"ow{l}")
                for t_sb, t_e in ((wq, wq_e), (wk, wk_e), (wv, wv_e),
                                  (ow, ow_e)):
                    nc.scalar.dma_start(
                        out=t_sb[:],
                        in_=t_e.ap()[l].rearrange("(a p) m -> p a m",
                                                  p=128))

                qT = at.tile([128, 2, T], F32R, name=f"qT{l}")
                kT = at.tile([128, 2, T], F32R, name=f"kT{l}")
                vN = at.tile([128, NSB, HPC * (HS + 1)], BF16,
                             name=f"vN{l}")
                vN4 = vN[:].rearrange("p s (h e) -> p s h e", e=HS + 1)
                oT = at.tile([128, 2, T], BF16, name=f"oT{l}")

                def qkv_half(tt, rstd_b, rstd_c):
                    tsl = slice(tt * 512, (tt + 1) * 512)
                    for dst, w_sb in ((qT, wq), (kT, wk)):
                        for mb in range(2):
                            pq = psum_big("pq")[:, 0:512]
                            for cb in range(NCB):
                                nc.tensor.matmul(
                                    pq,
                                    w_sb[:, cb, mb * 128:(mb + 1) * 128],
                                    xT[:, cb, tsl],
                                    start=(cb == 0),
                                    stop=(cb == NCB - 1))
                            nc.vector.tensor_mul(dst[:, mb, tsl], pq,
                                                 rstd_b[:])
                    for sb4 in range(4):
                        sb = tt * 4 + sb4
                        nc.vector.tensor_copy(vN4[:, sb, :, HS:HS + 1],
                                              ones_f[:, 0:HPC])
                        pv = psum_big("pv")[:, 0:512]
                        for cb in range(NCB):
                            nc.tensor.matmul(
                                pv[:, 0:HPC * HS],
                                xT[:, cb, sb * 128:(sb + 1) * 128],
                                wv[:, cb, :],
                                start=(cb == 0), stop=(cb == NCB - 1))
                        nc.vector.tensor_scalar_mul(
                            vN4[:, sb, :, 0:HS],
                            pv[:, 0:HPC * HS].rearrange(
                                "p (h e) -> p h e", e=HS),
                            rstd_c[:, sb4:sb4 + 1])

                def attn_tt(tt):
                    n_sb = (tt + 1) * 4
                    tsl = slice(tt * 512, (tt + 1) * 512)
                    for w in range(2):
                        avp = [sp.tile([128, 512], F32, tag=f"av{i}",
                                       name=f"av{i}") for i in range(4)]
                        pend_av = []

                        def do_av(sb, Ps):
                            first, last = (sb == 0), (sb == n_sb - 1)
                            for hp in range(2):
                                hh = 2 * w + hp
                                for qq in range(2):
                                    q = 2 * hp + qq
                                    nc.tensor.matmul(
                                        avp[q][0:65, :],
                                        vN[:, sb, hh * (HS + 1):
                                           hh * (HS + 1) + HS + 1],
                                        Ps[hp][:,
                                               qq * 512:(qq + 1) * 512],
                                        start=first, stop=last)

                        for sb in range(n_sb):
                            Ps = []
                            for hp in range(2):
                                scp = psum_big(f"sc{hp}")
                                for qq in range(2):
                                    q = 2 * hp + qq
                                    nc.tensor.matmul(
                                        scp[:, qq * 512:(qq + 1) * 512],
                                        kT[32 * q:32 * q + 32, w,
                                           sb * 128:(sb + 1) * 128],
                                        qT[32 * q:32 * q + 32, w, tsl],
                                        start=True, stop=True,
                                        tile_position=(32 * q, 0))
                                P = at.tile([128, 1024], BF16,
                                            tag=f"P{hp}", bufs=2,
                                            name=f"P{hp}")
                                nc.scalar.activation(P[:], scp[:],
                                                     AF.Exp)
                                m = sb - tt * 4
                                if m >= 0:
                                    nc.vector.tensor_mul(
                                        P[:], P[:], masks[:, m, :])
                                Ps.append(P)
                            pend_av.append((sb, Ps))
                            if len(pend_av) > 1:
                                do_av(*pend_av.pop(0))
                        do_av(*pend_av.pop(0))

                        # combine dual softmax -> oT
                        zr = at.tile([128, 512], F32R, tag="zr",
                                     name="zr")
                        bcs = []
                        for q in range(4):
                            nc.scalar.copy(zr[64:65, :],
                                           avp[q][64:65, :])
                            bcp = psum_big(f"bc{q}")[:, 0:512]
                            nc.tensor.matmul(
                                bcp, ones128[64:65, :], zr[64:65, :],
                                start=True, stop=True,
                                tile_position=(64, 0))
                            bc = at.tile([128, 512], F32,
                                         tag=f"bcs{q % 2}", bufs=1,
                                         name=f"bcsb{q}")
                            nc.vector.reciprocal_approx_fast(
                                out=bc[:], in_=bcp)
                            bcs.append(bc)
                        scr = at.tile([128, 512], F32, tag="scr",
                                      name="scr")
                        oshift = at.tile([128, 512], BF16, tag="osh",
                                         bufs=2, name="oshift")
                        for hl in range(2):
                            h = 2 * w + hl
                            dst = (oT[0:64, w, tsl] if hl == 0
                                   else oshift[0:64, :])
                            nc.vector.tensor_mul(
                                scr[0:64, :], avp[2 * hl + 1][0:64, :],
                                bcs[2 * hl + 1][0:64, :])
                            nc.vector.tensor_mul(
                                dst, avp[2 * hl][0:64, :],
                                bcs[2 * hl][0:64, :])
                            nc.vector.scalar_tensor_tensor(
                                out=dst, in0=scr[0:64, :],
                                scalar=neglam[0:64, l, h:h + 1],
                                in1=dst,
                                op0=ALU.mult, op1=ALU.add)
                            if hl == 1:
                                nc.sync.dma_start(
                                    out=oT[64:128, w, tsl],
                                    in_=oshift[0:64, :])

                if ar_mode == "full":
                    _ai = dram.tile([C, T], BF16, tag="ariAF",
                                    name=f"ariAF_{l}")
                    _ao = dram.tile([C, T], BF16, tag="aroAF",
                                    name=f"aroAF_{l}")
                    arA_in = [_ai[:, 0:512], _ai[:, 512:1024]]
                    arA_out = [_ao[:, 0:512], _ao[:, 512:1024]]
                    arA_full = (_ai, _ao)
                else:
                    arA_in = [dram.tile([C, 512], BF16, tag=f"ariA{tt}",
                                        name=f"ariA{tt}_{l}")[:]
                              for tt in range(2)]
                    arA_out = [dram.tile([C, 512], BF16,
                                         tag=f"aroA{tt}",
                                         name=f"aroA{tt}_{l}")[:]
                               for tt in range(2)]
                    arA_full = None

                def op_half(tt):
                    tsl = slice(tt * 512, (tt + 1) * 512)
                    for mb in range(NCB):
                        po = psum_big("po")[:, 0:512]
                        for kb in range(2):
                            nc.tensor.matmul(
                                po,
                                ow[:, kb, mb * 128:(mb + 1) * 128],
                                oT[:, kb, tsl],
                                start=(kb == 0), stop=(kb == 1))
                        yst = at.tile([128, 512], BF16, tag="yst",
                                      bufs=2, name="yst")
                        nc.vector.tensor_copy(yst[:], po)
                        nc.sync.dma_start(
                            out=arA_in[tt][mb * 128:(mb + 1) * 128, :],
                            in_=yst[:])
                    if no_collective:
                        nc.sync.dma_start(out=arA_out[tt],
                                          in_=arA_in[tt])
                    elif ar_mode == "full":
                        if tt == 1:
                            nc.gpsimd.collective_compute(
                                "AllReduce", ALU.add,
                                replica_groups=GROUPS,
                                ins=[arA_full[0].opt()],
                                outs=[arA_full[1].opt()])
                    else:
                        nc.gpsimd.collective_compute(
                            "AllReduce", ALU.add, replica_groups=GROUPS,
                            ins=[arA_in[tt].opt()],
                            outs=[arA_out[tt].opt()])

                # ---- emission schedule ----
                rstd_b0, rstd_c0 = stats_half(at, 0, "a")
                qkv_half(0, rstd_b0, rstd_c0)
                attn_tt(0)
                op_half(0)                      # -> AR(0)
                if pend_h1_final[0] is not None:
                    pend_h1_final[0]()          # resid M(l-1, h1)
                    pend_h1_final[0] = None
                rstd_b1, rstd_c1 = stats_half(at, 1, "a")
                qkv_half(1, rstd_b1, rstd_c1)
                attn_tt(1)                      # overlaps AR(0)
                op_half(1)                      # -> AR(1)
                at.release()

                # ======== MLP ========
                ml = tc.alloc_tile_pool(name=f"ml{_rep}_{l}", bufs=1, side="right")
                w1 = ml.tile([128, NCB, 2 * HIDS], F32R, name=f"w1{l}")
                w2 = ml.tile([128, NGB, C], BF16, name=f"w2{l}")
                nc.scalar.dma_start(
                    out=w1[:],
                    in_=w1_e.ap()[l].rearrange("(a p) m -> p a m",
                                               p=128))
                nc.scalar.dma_start(
                    out=w2[:],
                    in_=w2_e.ap()[l].rearrange("(a p) m -> p a m",
                                               p=128))
                if ar_mode == "full":
                    _mi = dram.tile([C, T], BF16, tag="ariMF",
                                    name=f"ariMF_{l}")
                    _mo = dram.tile([C, T], BF16, tag="aroMF",
                                    name=f"aroMF_{l}")
                    arM_in = [_mi[:, 0:512], _mi[:, 512:1024]]
                    arM_out = [_mo[:, 0:512], _mo[:, 512:1024]]
                    arM_full = (_mi, _mo)
                else:
                    arM_in = [dram.tile([C, 512], BF16, tag=f"ariM{tt}",
                                        name=f"ariM{tt}_{l}")[:]
                              for tt in range(2)]
                    arM_out = [dram.tile([C, 512], BF16,
                                         tag=f"aroM{tt}",
                                         name=f"aroM{tt}_{l}")[:]
                               for tt in range(2)]
                    arM_full = None

                def resid_half(tt, ar_out, tag, ar_in_=None):
                    if ar_mode == "nowait" and ar_in_ is not None:
                        ar_out = ar_in_
                    tsl = slice(tt * 512, (tt + 1) * 512)
                    for cb in range(NCB):
                        yf = ml.tile([128, 512], BF16, tag=f"yf{tag}",
                                     bufs=2, name=f"yf{tag}")
                        nc.sync.dma_start(
                            out=yf[:],
                            in_=ar_out[cb * 128:(cb + 1) * 128, :])
                        nc.vector.tensor_add(xT[:, cb, tsl],
                                             xT[:, cb, tsl], yf[:])

                def mlp_half(tt, rstd2b, gsb, asb):
                    tsl = slice(tt * 512, (tt + 1) * 512)
                    for gb in range(2 * NGB):
                        pu = psum_big(f"pu{gb}")[:, 0:512]
                        for cb in range(NCB):
                            nc.tensor.matmul(
                                pu,
                                w1[:, cb, gb * 128:(gb + 1) * 128],
                                xT[:, cb, tsl],
                                start=(cb == 0), stop=(cb == NCB - 1))
                        if gb < NGB:
                            nc.vector.tensor_mul(gsb[:, gb, :], pu,
                                                 rstd2b[:])
                        else:
                            nc.vector.tensor_mul(asb[:, gb - NGB, :],
                                                 pu, rstd2b[:])
                    for gb in range(NGB):
                        nc.scalar.activation(asb[:, gb, :],
                                             asb[:, gb, :], AF.Silu)
                        nc.vector.tensor_mul(gsb[:, gb, :],
                                             asb[:, gb, :],
                                             gsb[:, gb, :])
                    for mb in range(NCB):
                        pz = psum_big(f"pz{mb}")[:, 0:512]
                        for kb in range(NGB):
                            nc.tensor.matmul(
                                pz,
                                w2[:, kb, mb * 128:(mb + 1) * 128],
                                gsb[:, kb, :],
                                start=(kb == 0), stop=(kb == NGB - 1))
                        zst = ml.tile([128, 512], BF16, tag="zst",
                                      bufs=2, name="zst")
                        nc.vector.tensor_copy(zst[:], pz)
                        nc.sync.dma_start(
                            out=arM_in[tt][mb * 128:(mb + 1) * 128, :],
                            in_=zst[:])
                    if no_collective:
                        nc.sync.dma_start(out=arM_out[tt],
                                          in_=arM_in[tt])
                    elif ar_mode == "full":
                        if tt == 1:
                            nc.gpsimd.collective_compute(
                                "AllReduce", ALU.add,
                                replica_groups=GROUPS,
                                ins=[arM_full[0].opt()],
                                outs=[arM_full[1].opt()])
                    else:
                        nc.gpsimd.collective_compute(
                            "AllReduce", ALU.add, replica_groups=GROUPS,
                            ins=[arM_in[tt].opt()],
                            outs=[arM_out[tt].opt()])

                gsb = [ml.tile([128, NGB, 512], BF16, name=f"gsb{i}")
                       for i in range(2)]
                asb = [ml.tile([128, NGB, 512], BF16, name=f"asb{i}")
                       for i in range(2)]

                resid_half(0, arA_out[0], "a", arA_in[0])
                rstd2b0, _ = stats_half(ml, 0, "m", want_cols=False)
                mlp_half(0, rstd2b0, gsb[0], asb[0])    # -> AR2(0)
                resid_half(1, arA_out[1], "a", arA_in[1])
                rstd2b1, _ = stats_half(ml, 1, "m", want_cols=False)
                mlp_half(1, rstd2b1, gsb[1], asb[1])    # -> AR2(1)
                resid_half(0, arM_out[0], "m", arM_in[0])

                def make_final(ml_pool_, arM_out1_, arM_in1_):
                    def fin():
                        tsl = slice(512, 1024)
                        for cb in range(NCB):
                            zf = ml_pool_.tile([128, 512], BF16,
                                               tag="yfm", bufs=2,
                                               name="zf")
                            src_ = (arM_in1_ if ar_mode == "nowait"
                                    else arM_out1_)
                            nc.sync.dma_start(
                                out=zf[:],
                                in_=src_[cb * 128:
                                         (cb + 1) * 128, :])
                            nc.vector.tensor_add(xT[:, cb, tsl],
                                                 xT[:, cb, tsl],
                                                 zf[:])
                        ml_pool_.release()
                    return fin

                pend_h1_final[0] = make_final(ml, arM_out[1],
                                              arM_in[1])

            if pend_h1_final[0] is not None:
                pend_h1_final[0]()
                pend_h1_final[0] = None

            # ---------------- ln_f -> xc (bf16) ----------------
            hd = tc.alloc_tile_pool(name=f"hd{_rep}", bufs=1)
            xc = hd.tile([128, NCB, T], BF16, name="xc")
            hd0 = tc.alloc_tile_pool(name=f"hd0{_rep}", bufs=1)
            for tt in range(NT):
                tsl = slice(tt * 512, (tt + 1) * 512)
                big = psum_big("mu_ms")
                mu_b = big[:, 0:512]
                for cb in range(NCB):
                    nc.tensor.matmul(mu_b, ones128[:], xT[:, cb, tsl],
                                     start=(cb == 0),
                                     stop=(cb == NCB - 1))
                sqf = xpool.tile([128, NCB, 512], BF16, tag="sq",
                                 bufs=1, name="sqf")
                for cb in range(NCB):
                    nc.scalar.activation(sqf[:, cb, :], xT[:, cb, tsl],
                                         AF.Square)
                ms_b = big[:, 512:1024]
                for cb in range(NCB):
                    nc.tensor.matmul(ms_b, ones128b[:], sqf[:, cb, :],
                                     start=(cb == 0),
                                     stop=(cb == NCB - 1))
                negmu = hd0.tile([128, 512], F32, tag="negmu",
                                 name="negmu")
                nc.scalar.activation(negmu[:], mu_b, AF.Copy,
                                     scale=-1.0 / C)
                mom2 = hd0.tile([128, 512], F32, tag="mom2",
                                name="mom2")
                nc.scalar.activation(mom2[:], ms_b, AF.Copy,
                                     scale=1.0 / C)
                mu2 = hd0.tile([128, 512], F32, tag="mu2", name="mu2")
                nc.vector.tensor_mul(mu2[:], negmu[:], negmu[:])
                var = hd0.tile([128, 512], F32, tag="var", name="var")
                nc.vector.tensor_sub(var[:], mom2[:], mu2[:])
                stdf = hd0.tile([128, 512], F32, tag="stdf",
                                name="stdf")
                nc.scalar.activation(stdf[:], var[:], AF.Sqrt, bias=EPS)
                rstdf = hd0.tile([128, 512], F32, tag="rstdf",
                                 name="rstdf")
                nc.vector.reciprocal_approx_fast(out=rstdf[:],
                                                 in_=stdf[:])
                for cb in range(NCB):
                    xcen = hd0.tile([128, 512], F32, tag="xcen",
                                    bufs=2, name="xcen")
                    nc.vector.tensor_add(xcen[:], xT[:, cb, tsl],
                                         negmu[:])
                    nc.vector.tensor_mul(xc[:, cb, tsl], xcen[:],
                                         rstdf[:])
            hd0.release()

            # ---------------- lm_head (bf16) ----------------
            with tc.tile_pool(name=f"hw{_rep}", bufs=3) as hw:
                NVT = VSH // 512            # 25
                VG = 2
                n_groups = (NVT + VG - 1) // VG
                for g in range(n_groups):
                    vts = list(range(g * VG, min((g + 1) * VG, NVT)))
                    nv = len(vts)
                    wt = hw.tile([128, NCB, VG * 512], BF16, tag="wt",
                                 name=f"wt{g}")
                    nc.scalar.dma_start(
                        out=wt[:, :, 0:nv * 512],
                        in_=wteT_e.ap()[:, vts[0] * 512:
                                        (vts[-1] + 1) * 512]
                            .rearrange("(a p) v -> p a v", p=128))
                    for tb in range(NSB):
                        po = 2 * (tb % 2)
                        lps = [sp.tile([128, 512], F32,
                                       tag=f"av{po + i}",
                                       name=f"lg{g}_{tb}_{i}")
                               for i in range(nv)]
                        for cb in range(NCB):
                            for i in range(nv):
                                nc.tensor.matmul(
                                    lps[i][:],
                                    xc[:, cb,
                                       tb * 128:(tb + 1) * 128],
                                    wt[:, cb, i * 512:(i + 1) * 512],
                                    start=(cb == 0),
                                    stop=(cb == NCB - 1))
                        lo = hd.tile([128, VG * 512], BF16, tag="lo",
                                     bufs=3, name=f"lo{g}_{tb}")
                        for i in range(nv):
                            if i % 2 == 0:
                                nc.scalar.copy(
                                    lo[:, i * 512:(i + 1) * 512],
                                    lps[i][:])
                            else:
                                nc.vector.tensor_copy(
                                    lo[:, i * 512:(i + 1) * 512],
                                    lps[i][:])
                        nc.sync.dma_start(
                            out=logits_e[tb * 128:(tb + 1) * 128,
                                         vts[0] * 512:
                                         (vts[-1] + 1) * 512],
                            in_=lo[:, 0:nv * 512])
            hd.release()

          xpool.release()

    nc.compile()
    return nc


def _prep_inputs(inputs):
    import ml_dtypes
    idx = np.asarray(inputs["idx"]).astype(np.int64)
    wte = np.asarray(inputs["wte"], np.float32)
    wpe = np.asarray(inputs["wpe"], np.float32)
    rms1 = np.asarray(inputs["rms1_w"], np.float32)
    rms2 = np.asarray(inputs["rms2_w"], np.float32)
    wq = np.asarray(inputs["wq"], np.float32)
    wk = np.asarray(inputs["wk"], np.float32)
    wv = np.asarray(inputs["wv"], np.float32)
    lq1 = np.asarray(inputs["lq1"], np.float32)
    lq2 = np.asarray(inputs["lq2"], np.float32)
    lk1 = np.asarray(inputs["lk1"], np.float32)
    lk2 = np.asarray(inputs["lk2"], np.float32)
    out_w = np.asarray(inputs["out_w"], np.float32)
    out_b = np.asarray(inputs["out_b"], np.float32)
    mlp_w1 = np.asarray(inputs["mlp_w1"], np.float32)
    mlp_b1 = np.asarray(inputs["mlp_b1"], np.float32)
    cproj_w = np.asarray(inputs["cproj_w"], np.float32)
    cproj_b = np.asarray(inputs["cproj_b"], np.float32)
    lnf_w = np.asarray(inputs["lnf_w"], np.float32)

    assert not (np.any(out_b) or np.any(mlp_b1) or np.any(cproj_b)), \
        "nonzero biases not supported by this kernel build"

    depth = np.arange(L, dtype=np.float32)
    lam_init = 0.8 - 0.6 * np.exp(-0.3 * (depth - 1.0))
    lam = (np.exp((lq1 * lk1).sum(-1)) - np.exp((lq2 * lk2).sum(-1))
           + lam_init[:, None])

    wteE = wte[:BLK] + wpe
    scale = 1.0 / math.sqrt(D)
    wq_f = wq * rms1[:, :, None, None] * scale
    wk_f = wk * rms1[:, :, None, None]
    wv_f = wv * rms1[:, :, None, None]
    w1_f = mlp_w1 * rms2[:, :, None]
    wteT_full = np.ascontiguousarray((wte * lnf_w[None, :]).T)

    jj = np.arange(512)[None, :]
    ppp = np.arange(128)[:, None]
    masks = np.zeros((128, 4, 2048), np.float32)
    for m in range(4):
        one = (jj >= (ppp + 128 * m)).astype(np.float32)
        masks[:, m, :] = np.tile(one, (1, 4))
    masks = masks.astype(ml_dtypes.bfloat16)

    in_maps = []
    for c in range(N_CORES):
        b, r = c // TP, c % TP
        hsl = slice(r * HPC, (r + 1) * HPC)
        x0 = np.ascontiguousarray(wteE[idx[b]].T)
        g0 = r * HIDS
        a0 = 2 * C + r * HIDS
        w1_s = np.concatenate(
            [w1_f[:, :, g0:g0 + HIDS], w1_f[:, :, a0:a0 + HIDS]],
            axis=2)
        w2_s = cproj_w[:, g0:g0 + HIDS, :]
        wteT_s = np.zeros((C, VSH), np.float32)
        lo, hi = r * VSH, min((r + 1) * VSH, V)
        if hi > lo:
            wteT_s[:, 0:hi - lo] = wteT_full[:, lo:hi]
        in_maps.append({
            "x0": x0,
            "wq": np.ascontiguousarray(
                wq_f[:, :, hsl].reshape(L, C, HPC * HS)),
            "wk": np.ascontiguousarray(
                wk_f[:, :, hsl].reshape(L, C, HPC * HS)),
            "wv": np.ascontiguousarray(
                wv_f[:, :, hsl].reshape(L, C, HPC * HS)),
            "ow": np.ascontiguousarray(
                out_w.reshape(L, H, HS, C)[:, hsl].reshape(
                    L, HPC * HS, C)).astype(ml_dtypes.bfloat16),
            "w1": np.ascontiguousarray(w1_s),
            "w2": np.ascontiguousarray(w2_s).astype(ml_dtypes.bfloat16),
            "neglam": np.ascontiguousarray(-lam[:, hsl]),
            "masks": masks,
            "wteT": wteT_s.astype(ml_dtypes.bfloat16),
        })
    return in_maps


def kernel(**inputs):
    from concourse.bass_utils import run_bass_kernel_spmd
    if "nc" not in _BUILT:
        _BUILT["nc"] = _build()
    nc = _BUILT["nc"]
    in_maps = _prep_inputs(inputs)
    res = run_bass_kernel_spmd(nc, in_maps, core_ids=list(range(N_CORES)))
    outs = []
    for b in range(B):
        parts = [res.results[b * TP + r]["logits"]
                 for r in range(TP)]
        outs.append(np.concatenate(parts, axis=1)[:, :V])
    return np.stack(outs, axis=0).astype(np.float32)


# revision 24
# speedup vs baseline: 47.0864x; 1.0142x over previous
"""DifferentialGPT forward on 8 TRN2 NeuronCores (Bass/Tile).

DP2 over batch x TP4 (4 heads, 512 MLP pairs, 12800 vocab cols per core).
bf16 AllReduce (groups of 4) after out_proj and c_proj, split into T-halves
and software-pipelined against compute, including across the layer boundary
(layer l+1 half-0 stats/QKV/attention overlap layer l's last AllReduce).
Residual kept transposed xT [C(part), T(free)] in f32r; moving-side matmuls
f32r or bf16; lm_head bf16 with bf16 logits output (host casts to f32).
Bulk weight DMAs ride the Activation-engine HWDGE queue, AR staging and
output stores ride the SP queue.
"""
import math
import numpy as np

V, BLK, C, H, L = 50257, 1024, 1024, 16, 4
B, T = 2, 1024
HS = C // H          # 64
D = HS // 2          # 32
EPS = 1e-5
N_CORES = 8
TP = 4
HPC = H // TP        # 4
VSH = 12800
HIDS = 4 * C // TP // 2   # 512
NCB = C // 128       # 8
NT = T // 512        # 2
NSB = T // 128       # 8
NGB = HIDS // 128    # 4

_BUILT = {}


def _build(rep_count=1, no_collective=False, ar_mode="half"):
    # ar_mode: "half"  - one AllReduce per T-half (default)
    #          "full"  - one AllReduce per sublayer over full T
    #          "nowait"- half ARs issued, but residual reads the un-reduced
    #                    input (WRONG results; timing diagnostic only)
    from concourse import bass, mybir, bacc
    import concourse.tile as tile

    F32 = mybir.dt.float32
    F32R = mybir.dt.float32r
    BF16 = mybir.dt.bfloat16
    AF = mybir.ActivationFunctionType
    ALU = mybir.AluOpType

    nc = bacc.Bacc("TRN2", target_bir_lowering=False, debug=False,
                   num_devices=N_CORES)
    for _cv in (EPS,):
        _ct = nc.alloc_sbuf_tensor(f"const-float32-{_cv}", [128, 1], F32)
        nc.gpsimd.memset(_ct.ap(), _cv)
        nc.const_aps.aps[(F32, _cv)] = _ct.ap()
    nc.all_engine_barrier()

    def EIN(name, shape, dt):
        return nc.dram_tensor(name, shape, dt, kind="ExternalInput")

    x0_e = EIN("x0", [C, T], F32R)
    wq_e = EIN("wq", [L, C, HPC * HS], F32R)
    wk_e = EIN("wk", [L, C, HPC * HS], F32R)
    wv_e = EIN("wv", [L, C, HPC * HS], F32R)
    ow_e = EIN("ow", [L, HPC * HS, C], BF16)
    w1_e = EIN("w1", [L, C, 2 * HIDS], F32R)
    w2_e = EIN("w2", [L, HIDS, C], BF16)
    neglam_e = EIN("neglam", [L, HPC], F32)
    masks_e = EIN("masks", [128, 4, 2048], BF16)
    wteT_e = EIN("wteT", [C, VSH], BF16)
    logits_e = nc.dram_tensor("logits", [T, VSH], BF16,
                              kind="ExternalOutput")

    GROUPS = [[0, 1, 2, 3], [4, 5, 6, 7]]

    with tile.TileContext(nc) as tc:
      with (
        tc.tile_pool(name="persist", bufs=1) as pp,
        tc.tile_pool(name="dram", bufs=1, space="DRAM") as dram,
      ):
        masks = pp.tile([128, 4, 1024], BF16, name="masks")
        nc.scalar.dma_start(out=masks[:], in_=masks_e[:, :, 0:1024])
        neglam = pp.tile([128, L, HPC], F32, name="neglam")
        nc.scalar.dma_start(
            out=neglam[:],
            in_=bass.AP(tensor=neglam_e, offset=0,
                        ap=[[0, 128], [HPC, L], [1, HPC]]))
        ones_f = pp.tile([128, 128], F32, name="ones_f")
        nc.vector.memset(ones_f[:], 1.0)
        ones128 = pp.tile([128, 128], F32R, name="ones128")
        nc.vector.tensor_copy(ones128[:], ones_f[:])
        ones128b = pp.tile([128, 128], BF16, name="ones128b")
        nc.vector.tensor_copy(ones128b[:], ones_f[:])
        ones_cb = ones128b[:, 0:1]

        for _rep in range(rep_count):
          xpool = tc.alloc_tile_pool(name=f"xpool{_rep}", bufs=1)
          xT = xpool.tile([128, NCB, T], F32R, name="xT")
          for tt in range(NT):
              tsl = slice(tt * 512, (tt + 1) * 512)
              nc.scalar.dma_start(
                  out=xT[:, :, tsl],
                  in_=x0_e.ap()[:, tsl].rearrange("(a p) t -> p a t",
                                                  p=128))

          with tc.tile_pool(name=f"ps{_rep}", bufs=1, space="PSUM") as sp:

            def psum_big(name):
                return sp.tile([128, 1024], F32, tag="sc", bufs=2,
                               name=name)

            # rmsnorm stats for half tt: rstd broadcast [128,512] and
            # per-token rstd columns [128,4] (token in partition, by sb)
            def stats_half(pool, tt, tag, want_cols=True):
                tsl = slice(tt * 512, (tt + 1) * 512)
                sq = xpool.tile([128, NCB, 512], BF16, tag="sq", bufs=1,
                                name=f"sq{tag}{tt}")
                for cb in range(NCB):
                    nc.scalar.activation(sq[:, cb, :], xT[:, cb, tsl],
                                         AF.Square)
                big = psum_big(f"ssq{tag}{tt}")
                ssq_b = big[:, 0:512]
                for cb in range(NCB):
                    nc.tensor.matmul(ssq_b, ones128b[:], sq[:, cb, :],
                                     start=(cb == 0),
                                     stop=(cb == NCB - 1))
                if want_cols:
                    for sb4 in range(4):
                        for cb in range(NCB):
                            nc.tensor.matmul(
                                big[:, 512 + sb4:513 + sb4],
                                sq[:, cb, sb4 * 128:(sb4 + 1) * 128],
                                ones_cb,
                                start=(cb == 0), stop=(cb == NCB - 1))
                std_b = pool.tile([128, 512], F32, tag=f"sdb{tag}",
                                  name=f"sdb{tag}{tt}")
                nc.scalar.activation(std_b[:], ssq_b, AF.Sqrt,
                                     scale=1.0 / C, bias=EPS)
                rstd_b = pool.tile([128, 512], F32, tag=f"rsb{tag}",
                                   name=f"rsb{tag}{tt}")
                nc.vector.reciprocal_approx_fast(out=rstd_b[:],
                                                 in_=std_b[:])
                rstd_c = None
                if want_cols:
                    std_c = pool.tile([128, 8], F32, tag=f"sdc{tag}",
                                      name=f"sdc{tag}{tt}")
                    nc.scalar.activation(std_c[:, 0:4],
                                         big[:, 512:516], AF.Sqrt,
                                         scale=1.0 / C, bias=EPS)
                    rstd_c = pool.tile([128, 8], F32, tag=f"rsc{tag}",
                                       name=f"rsc{tag}{tt}")
                    nc.vector.reciprocal_approx_fast(
                        out=rstd_c[:, 0:4], in_=std_c[:, 0:4])
                return rstd_b, rstd_c

            pend_h1_final = [None]

            for l in range(L):
                at = tc.alloc_tile_pool(name=f"at{_rep}_{l}", bufs=1)
                wq = at.tile([128, NCB, HPC * HS], F32R, name=f"wq{l}")
                wk = at.tile([128, NCB, HPC * HS], F32R, name=f"wk{l}")
                wv = at.tile([128, NCB, HPC * HS], F32R, name=f"wv{l}")
                ow = at.tile([128, (HPC * HS) // 128, C], BF16,
                             name=f"ow{l}")
                for t_sb, t_e in ((wq, wq_e), (wk, wk_e), (wv, wv_e),
                                  (ow, ow_e)):
                    nc.scalar.dma_start(
                        out=t_sb[:],
                        in_=t_e.ap()[l].rearrange("(a p) m -> p a m",
                                                  p=128))

                qT = at.tile([128, 2, T], F32R, name=f"qT{l}")
                kT = at.tile([128, 2, T], F32R, name=f"kT{l}")
                vN = at.tile([128, NSB, HPC * (HS + 1)], BF16,
                             name=f"vN{l}")
                vN4 = vN[:].rearrange("p s (h e) -> p s h e", e=HS + 1)
                oT = at.tile([128, 2, T], BF16, name=f"oT{l}")

                def qkv_half(tt, rstd_b, rstd_c):
                    tsl = slice(tt * 512, (tt + 1) * 512)
                    for dst, w_sb in ((qT, wq), (kT, wk)):
                        for mb in range(2):
                            pq = psum_big("pq")[:, 0:512]
                            for cb in range(NCB):
                                nc.tensor.matmul(
                                    pq,
                                    w_sb[:, cb, mb * 128:(mb + 1) * 128],
                                    xT[:, cb, tsl],
                                    start=(cb == 0),
                                    stop=(cb == NCB - 1))
                            nc.vector.tensor_mul(dst[:, mb, tsl], pq,
                                                 rstd_b[:])
                    for sb4 in range(4):
                        sb = tt * 4 + sb4
                        nc.vector.tensor_copy(vN4[:, sb, :, HS:HS + 1],
                                              ones_f[:, 0:HPC])
                        pv = psum_big("pv")[:, 0:512]
                        for cb in range(NCB):
                            nc.tensor.matmul(
                                pv[:, 0:HPC * HS],
                                xT[:, cb, sb * 128:(sb + 1) * 128],
                                wv[:, cb, :],
                                start=(cb == 0), stop=(cb == NCB - 1))
                        nc.vector.tensor_scalar_mul(
                            vN4[:, sb, :, 0:HS],
                            pv[:, 0:HPC * HS].rearrange(
                                "p (h e) -> p h e", e=HS),
                            rstd_c[:, sb4:sb4 + 1])

                def attn_tt(tt):
                    n_sb = (tt + 1) * 4
                    tsl = slice(tt * 512, (tt + 1) * 512)
                    for w in range(2):
                        avp = [sp.tile([128, 512], F32, tag=f"av{i}",
                                       name=f"av{i}") for i in range(4)]
                        pend_av = []

                        def do_av(sb, Ps):
                            first, last = (sb == 0), (sb == n_sb - 1)
                            for hp in range(2):
                                hh = 2 * w + hp
                                for qq in range(2):
                                    q = 2 * hp + qq
                                    nc.tensor.matmul(
                                        avp[q][0:65, :],
                                        vN[:, sb, hh * (HS + 1):
                                           hh * (HS + 1) + HS + 1],
                                        Ps[hp][:,
                                               qq * 512:(qq + 1) * 512],
                                        start=first, stop=last)

                        for sb in range(n_sb):
                            Ps = []
                            for hp in range(2):
                                scp = psum_big(f"sc{hp}")
                                for qq in range(2):
                                    q = 2 * hp + qq
                                    nc.tensor.matmul(
                                        scp[:, qq * 512:(qq + 1) * 512],
                                        kT[32 * q:32 * q + 32, w,
                                           sb * 128:(sb + 1) * 128],
                                        qT[32 * q:32 * q + 32, w, tsl],
                                        start=True, stop=True,
                                        tile_position=(32 * q, 0))
                                P = at.tile([128, 1024], BF16,
                                            tag=f"P{hp}", bufs=2,
                                            name=f"P{hp}")
                                nc.scalar.activation(P[:], scp[:],
                                                     AF.Exp)
                                m = sb - tt * 4
                                if m >= 0:
                                    nc.vector.tensor_mul(
                                        P[:], P[:], masks[:, m, :])
                                Ps.append(P)
                            pend_av.append((sb, Ps))
                            if len(pend_av) > 1:
                                do_av(*pend_av.pop(0))
                        do_av(*pend_av.pop(0))

                        # combine dual softmax -> oT
                        zr = at.tile([128, 512], F32R, tag="zr",
                                     name="zr")
                        bcs = []
                        for q in range(4):
                            nc.scalar.copy(zr[64:65, :],
                                           avp[q][64:65, :])
                            bcp = psum_big(f"bc{q}")[:, 0:512]
                            nc.tensor.matmul(
                                bcp, ones128[64:65, :], zr[64:65, :],
                                start=True, stop=True,
                                tile_position=(64, 0))
                            bc = at.tile([128, 512], F32,
                                         tag=f"bcs{q % 2}", bufs=1,
                                         name=f"bcsb{q}")
                            nc.vector.reciprocal_approx_fast(
                                out=bc[:], in_=bcp)
                            bcs.append(bc)
                        scr = at.tile([128, 512], F32, tag="scr",
                                      name="scr")
                        oshift = at.tile([128, 512], BF16, tag="osh",
                                         bufs=2, name="oshift")
                        for hl in range(2):
                            h = 2 * w + hl
                            dst = (oT[0:64, w, tsl] if hl == 0
                                   else oshift[0:64, :])
                            nc.vector.tensor_mul(
                                scr[0:64, :], avp[2 * hl + 1][0:64, :],
                                bcs[2 * hl + 1][0:64, :])
                            nc.vector.tensor_mul(
                                dst, avp[2 * hl][0:64, :],
                                bcs[2 * hl][0:64, :])
                            nc.vector.scalar_tensor_tensor(
                                out=dst, in0=scr[0:64, :],
                                scalar=neglam[0:64, l, h:h + 1],
                                in1=dst,
                                op0=ALU.mult, op1=ALU.add)
                            if hl == 1:
                                nc.sync.dma_start(
                                    out=oT[64:128, w, tsl],
                                    in_=oshift[0:64, :])

                if ar_mode == "full":
                    _ai = dram.tile([C, T], BF16, tag="ariAF",
                                    name=f"ariAF_{l}")
                    _ao = dram.tile([C, T], BF16, tag="aroAF",
                                    name=f"aroAF_{l}")
                    arA_in = [_ai[:, 0:512], _ai[:, 512:1024]]
                    arA_out = [_ao[:, 0:512], _ao[:, 512:1024]]
                    arA_full = (_ai, _ao)
                else:
                    arA_in = [dram.tile([C, 512], BF16, tag=f"ariA{tt}",
                                        name=f"ariA{tt}_{l}")[:]
                              for tt in range(2)]
                    arA_out = [dram.tile([C, 512], BF16,
                                         tag=f"aroA{tt}",
                                         name=f"aroA{tt}_{l}")[:]
                               for tt in range(2)]
                    arA_full = None

                def op_half(tt):
                    tsl = slice(tt * 512, (tt + 1) * 512)
                    for mb in range(NCB):
                        po = psum_big("po")[:, 0:512]
                        for kb in range(2):
                            nc.tensor.matmul(
                                po,
                                ow[:, kb, mb * 128:(mb + 1) * 128],
                                oT[:, kb, tsl],
                                start=(kb == 0), stop=(kb == 1))
                        yst = at.tile([128, 512], BF16, tag="yst",
                                      bufs=2, name="yst")
                        nc.scalar.copy(yst[:], po)
                        nc.sync.dma_start(
                            out=arA_in[tt][mb * 128:(mb + 1) * 128, :],
                            in_=yst[:])
                    if no_collective:
                        nc.sync.dma_start(out=arA_out[tt],
                                          in_=arA_in[tt])
                    elif ar_mode == "full":
                        if tt == 1:
                            nc.gpsimd.collective_compute(
                                "AllReduce", ALU.add,
                                replica_groups=GROUPS,
                                ins=[arA_full[0].opt()],
                                outs=[arA_full[1].opt()])
                    else:
                        nc.gpsimd.collective_compute(
                            "AllReduce", ALU.add, replica_groups=GROUPS,
                            ins=[arA_in[tt].opt()],
                            outs=[arA_out[tt].opt()])

                # ---- emission schedule ----
                rstd_b0, rstd_c0 = stats_half(at, 0, "a")
                qkv_half(0, rstd_b0, rstd_c0)
                attn_tt(0)
                op_half(0)                      # -> AR(0)
                if pend_h1_final[0] is not None:
                    pend_h1_final[0]()          # resid M(l-1, h1)
                    pend_h1_final[0] = None
                rstd_b1, rstd_c1 = stats_half(at, 1, "a")
                qkv_half(1, rstd_b1, rstd_c1)
                attn_tt(1)                      # overlaps AR(0)
                op_half(1)                      # -> AR(1)
                at.release()

                # ======== MLP ========
                ml = tc.alloc_tile_pool(name=f"ml{_rep}_{l}", bufs=1, side="right")
                w1 = ml.tile([128, NCB, 2 * HIDS], F32R, name=f"w1{l}")
                w2 = ml.tile([128, NGB, C], BF16, name=f"w2{l}")
                nc.scalar.dma_start(
                    out=w1[:],
                    in_=w1_e.ap()[l].rearrange("(a p) m -> p a m",
                                               p=128))
                nc.scalar.dma_start(
                    out=w2[:],
                    in_=w2_e.ap()[l].rearrange("(a p) m -> p a m",
                                               p=128))
                if ar_mode == "full":
                    _mi = dram.tile([C, T], BF16, tag="ariMF",
                                    name=f"ariMF_{l}")
                    _mo = dram.tile([C, T], BF16, tag="aroMF",
                                    name=f"aroMF_{l}")
                    arM_in = [_mi[:, 0:512], _mi[:, 512:1024]]
                    arM_out = [_mo[:, 0:512], _mo[:, 512:1024]]
                    arM_full = (_mi, _mo)
                else:
                    arM_in = [dram.tile([C, 512], BF16, tag=f"ariM{tt}",
                                        name=f"ariM{tt}_{l}")[:]
                              for tt in range(2)]
                    arM_out = [dram.tile([C, 512], BF16,
                                         tag=f"aroM{tt}",
                                         name=f"aroM{tt}_{l}")[:]
                               for tt in range(2)]
                    arM_full = None

                def resid_half(tt, ar_out, tag, ar_in_=None):
                    if ar_mode == "nowait" and ar_in_ is not None:
                        ar_out = ar_in_
                    tsl = slice(tt * 512, (tt + 1) * 512)
                    for cb in range(NCB):
                        yf = ml.tile([128, 512], BF16, tag=f"yf{tag}",
                                     bufs=2, name=f"yf{tag}")
                        nc.sync.dma_start(
                            out=yf[:],
                            in_=ar_out[cb * 128:(cb + 1) * 128, :])
                        nc.vector.tensor_add(xT[:, cb, tsl],
                                             xT[:, cb, tsl], yf[:])

                def mlp_half(tt, rstd2b, gsb, asb):
                    tsl = slice(tt * 512, (tt + 1) * 512)
                    for gb in range(2 * NGB):
                        pu = psum_big(f"pu{gb}")[:, 0:512]
                        for cb in range(NCB):
                            nc.tensor.matmul(
                                pu,
                                w1[:, cb, gb * 128:(gb + 1) * 128],
                                xT[:, cb, tsl],
                                start=(cb == 0), stop=(cb == NCB - 1))
                        if gb < NGB:
                            nc.vector.tensor_mul(gsb[:, gb, :], pu,
                                                 rstd2b[:])
                        else:
                            nc.vector.tensor_mul(asb[:, gb - NGB, :],
                                                 pu, rstd2b[:])
                    for gb in range(NGB):
                        nc.scalar.activation(asb[:, gb, :],
                                             asb[:, gb, :], AF.Silu)
                        nc.vector.tensor_mul(gsb[:, gb, :],
                                             asb[:, gb, :],
                                             gsb[:, gb, :])
                    for mb in range(NCB):
                        pz = psum_big(f"pz{mb}")[:, 0:512]
                        for kb in range(NGB):
                            nc.tensor.matmul(
                                pz,
                                w2[:, kb, mb * 128:(mb + 1) * 128],
                                gsb[:, kb, :],
                                start=(kb == 0), stop=(kb == NGB - 1))
                        zst = ml.tile([128, 512], BF16, tag="zst",
                                      bufs=2, name="zst")
                        nc.vector.tensor_copy(zst[:], pz)
                        nc.sync.dma_start(
                            out=arM_in[tt][mb * 128:(mb + 1) * 128, :],
                            in_=zst[:])
                    if no_collective:
                        nc.sync.dma_start(out=arM_out[tt],
                                          in_=arM_in[tt])
                    elif ar_mode == "full":
                        if tt == 1:
                            nc.gpsimd.collective_compute(
                                "AllReduce", ALU.add,
                                replica_groups=GROUPS,
                                ins=[arM_full[0].opt()],
                                outs=[arM_full[1].opt()])
                    else:
                        nc.gpsimd.collective_compute(
                            "AllReduce", ALU.add, replica_groups=GROUPS,
                            ins=[arM_in[tt].opt()],
                            outs=[arM_out[tt].opt()])

                gsb = [ml.tile([128, NGB, 512], BF16, name=f"gsb{i}")
                       for i in range(2)]
                asb = [ml.tile([128, NGB, 512], BF16, name=f"asb{i}")
                       for i in range(2)]

                resid_half(0, arA_out[0], "a", arA_in[0])
                rstd2b0, _ = stats_half(ml, 0, "m", want_cols=False)
                mlp_half(0, rstd2b0, gsb[0], asb[0])    # -> AR2(0)
                resid_half(1, arA_out[1], "a", arA_in[1])
                rstd2b1, _ = stats_half(ml, 1, "m", want_cols=False)
                mlp_half(1, rstd2b1, gsb[1], asb[1])    # -> AR2(1)
                resid_half(0, arM_out[0], "m", arM_in[0])

                def make_final(ml_pool_, arM_out1_, arM_in1_):
                    def fin():
                        tsl = slice(512, 1024)
                        for cb in range(NCB):
                            zf = ml_pool_.tile([128, 512], BF16,
                                               tag="yfm", bufs=2,
                                               name="zf")
                            src_ = (arM_in1_ if ar_mode == "nowait"
                                    else arM_out1_)
                            nc.sync.dma_start(
                                out=zf[:],
                                in_=src_[cb * 128:
                                         (cb + 1) * 128, :])
                            nc.vector.tensor_add(xT[:, cb, tsl],
                                                 xT[:, cb, tsl],
                                                 zf[:])
                        ml_pool_.release()
                    return fin

                pend_h1_final[0] = make_final(ml, arM_out[1],
                                              arM_in[1])

            if pend_h1_final[0] is not None:
                pend_h1_final[0]()
                pend_h1_final[0] = None

            # ---------------- ln_f -> xc (bf16) ----------------
            hd = tc.alloc_tile_pool(name=f"hd{_rep}", bufs=1)
            xc = hd.tile([128, NCB, T], BF16, name="xc")
            hd0 = tc.alloc_tile_pool(name=f"hd0{_rep}", bufs=1)
            for tt in range(NT):
                tsl = slice(tt * 512, (tt + 1) * 512)
                big = psum_big("mu_ms")
                mu_b = big[:, 0:512]
                for cb in range(NCB):
                    nc.tensor.matmul(mu_b, ones128[:], xT[:, cb, tsl],
                                     start=(cb == 0),
                                     stop=(cb == NCB - 1))
                sqf = xpool.tile([128, NCB, 512], BF16, tag="sq",
                                 bufs=1, name="sqf")
                for cb in range(NCB):
                    nc.scalar.activation(sqf[:, cb, :], xT[:, cb, tsl],
                                         AF.Square)
                ms_b = big[:, 512:1024]
                for cb in range(NCB):
                    nc.tensor.matmul(ms_b, ones128b[:], sqf[:, cb, :],
                                     start=(cb == 0),
                                     stop=(cb == NCB - 1))
                negmu = hd0.tile([128, 512], F32, tag="negmu",
                                 name="negmu")
                nc.scalar.activation(negmu[:], mu_b, AF.Copy,
                                     scale=-1.0 / C)
                mom2 = hd0.tile([128, 512], F32, tag="mom2",
                                name="mom2")
                nc.scalar.activation(mom2[:], ms_b, AF.Copy,
                                     scale=1.0 / C)
                mu2 = hd0.tile([128, 512], F32, tag="mu2", name="mu2")
                nc.vector.tensor_mul(mu2[:], negmu[:], negmu[:])
                var = hd0.tile([128, 512], F32, tag="var", name="var")
                nc.vector.tensor_sub(var[:], mom2[:], mu2[:])
                stdf = hd0.tile([128, 512], F32, tag="stdf",
                                name="stdf")
                nc.scalar.activation(stdf[:], var[:], AF.Sqrt, bias=EPS)
                rstdf = hd0.tile([128, 512], F32, tag="rstdf",
                                 name="rstdf")
                nc.vector.reciprocal_approx_fast(out=rstdf[:],
                                                 in_=stdf[:])
                for cb in range(NCB):
                    xcen = hd0.tile([128, 512], F32, tag="xcen",
                                    bufs=2, name="xcen")
                    nc.vector.tensor_add(xcen[:], xT[:, cb, tsl],
                                         negmu[:])
                    nc.vector.tensor_mul(xc[:, cb, tsl], xcen[:],
                                         rstdf[:])
            hd0.release()

            # ---------------- lm_head (bf16) ----------------
            with tc.tile_pool(name=f"hw{_rep}", bufs=3) as hw:
                NVT = VSH // 512            # 25
                VG = 2
                n_groups = (NVT + VG - 1) // VG
                for g in range(n_groups):
                    vts = list(range(g * VG, min((g + 1) * VG, NVT)))
                    nv = len(vts)
                    wt = hw.tile([128, NCB, VG * 512], BF16, tag="wt",
                                 name=f"wt{g}")
                    nc.scalar.dma_start(
                        out=wt[:, :, 0:nv * 512],
                        in_=wteT_e.ap()[:, vts[0] * 512:
                                        (vts[-1] + 1) * 512]
                            .rearrange("(a p) v -> p a v", p=128))
                    for tb in range(NSB):
                        po = 2 * (tb % 2)
                        lps = [sp.tile([128, 512], F32,
                                       tag=f"av{po + i}",
                                       name=f"lg{g}_{tb}_{i}")
                               for i in range(nv)]
                        for cb in range(NCB):
                            for i in range(nv):
                                nc.tensor.matmul(
                                    lps[i][:],
                                    xc[:, cb,
                                       tb * 128:(tb + 1) * 128],
                                    wt[:, cb, i * 512:(i + 1) * 512],
                                    start=(cb == 0),
                                    stop=(cb == NCB - 1))
                        lo = hd.tile([128, VG * 512], BF16, tag="lo",
                                     bufs=3, name=f"lo{g}_{tb}")
                        for i in range(nv):
                            if i % 2 == 0:
                                nc.scalar.copy(
                                    lo[:, i * 512:(i + 1) * 512],
                                    lps[i][:])
                            else:
                                nc.vector.tensor_copy(
                                    lo[:, i * 512:(i + 1) * 512],
                                    lps[i][:])
                        nc.sync.dma_start(
                            out=logits_e[tb * 128:(tb + 1) * 128,
                                         vts[0] * 512:
                                         (vts[-1] + 1) * 512],
                            in_=lo[:, 0:nv * 512])
            hd.release()

          xpool.release()

    nc.compile()
    return nc


def _prep_inputs(inputs):
    import ml_dtypes
    idx = np.asarray(inputs["idx"]).astype(np.int64)
    wte = np.asarray(inputs["wte"], np.float32)
    wpe = np.asarray(inputs["wpe"], np.float32)
    rms1 = np.asarray(inputs["rms1_w"], np.float32)
    rms2 = np.asarray(inputs["rms2_w"], np.float32)
    wq = np.asarray(inputs["wq"], np.float32)
    wk = np.asarray(inputs["wk"], np.float32)
    wv = np.asarray(inputs["wv"], np.float32)
    lq1 = np.asarray(inputs["lq1"], np.float32)
    lq2 = np.asarray(inputs["lq2"], np.float32)
    lk1 = np.asarray(inputs["lk1"], np.float32)
    lk2 = np.asarray(inputs["lk2"], np.float32)
    out_w = np.asarray(inputs["out_w"], np.float32)
    out_b = np.asarray(inputs["out_b"], np.float32)
    mlp_w1 = np.asarray(inputs["mlp_w1"], np.float32)
    mlp_b1 = np.asarray(inputs["mlp_b1"], np.float32)
    cproj_w = np.asarray(inputs["cproj_w"], np.float32)
    cproj_b = np.asarray(inputs["cproj_b"], np.float32)
    lnf_w = np.asarray(inputs["lnf_w"], np.float32)

    assert not (np.any(out_b) or np.any(mlp_b1) or np.any(cproj_b)), \
        "nonzero biases not supported by this kernel build"

    depth = np.arange(L, dtype=np.float32)
    lam_init = 0.8 - 0.6 * np.exp(-0.3 * (depth - 1.0))
    lam = (np.exp((lq1 * lk1).sum(-1)) - np.exp((lq2 * lk2).sum(-1))
           + lam_init[:, None])

    wteE = wte[:BLK] + wpe
    scale = 1.0 / math.sqrt(D)
    wq_f = wq * rms1[:, :, None, None] * scale
    wk_f = wk * rms1[:, :, None, None]
    wv_f = wv * rms1[:, :, None, None]
    w1_f = mlp_w1 * rms2[:, :, None]
    wteT_full = np.ascontiguousarray((wte * lnf_w[None, :]).T)

    jj = np.arange(512)[None, :]
    ppp = np.arange(128)[:, None]
    masks = np.zeros((128, 4, 2048), np.float32)
    for m in range(4):
        one = (jj >= (ppp + 128 * m)).astype(np.float32)
        masks[:, m, :] = np.tile(one, (1, 4))
    masks = masks.astype(ml_dtypes.bfloat16)

    in_maps = []
    for c in range(N_CORES):
        b, r = c // TP, c % TP
        hsl = slice(r * HPC, (r + 1) * HPC)
        x0 = np.ascontiguousarray(wteE[idx[b]].T)
        g0 = r * HIDS
        a0 = 2 * C + r * HIDS
        w1_s = np.concatenate(
            [w1_f[:, :, g0:g0 + HIDS], w1_f[:, :, a0:a0 + HIDS]],
            axis=2)
        w2_s = cproj_w[:, g0:g0 + HIDS, :]
        wteT_s = np.zeros((C, VSH), np.float32)
        lo, hi = r * VSH, min((r + 1) * VSH, V)
        if hi > lo:
            wteT_s[:, 0:hi - lo] = wteT_full[:, lo:hi]
        in_maps.append({
            "x0": x0,
            "wq": np.ascontiguousarray(
                wq_f[:, :, hsl].reshape(L, C, HPC * HS)),
            "wk": np.ascontiguousarray(
                wk_f[:, :, hsl].reshape(L, C, HPC * HS)),
            "wv": np.ascontiguousarray(
                wv_f[:, :, hsl].reshape(L, C, HPC * HS)),
            "ow": np.ascontiguousarray(
                out_w.reshape(L, H, HS, C)[:, hsl].reshape(
                    L, HPC * HS, C)).astype(ml_dtypes.bfloat16),
            "w1": np.ascontiguousarray(w1_s),
            "w2": np.ascontiguousarray(w2_s).astype(ml_dtypes.bfloat16),
            "neglam": np.ascontiguousarray(-lam[:, hsl]),
            "masks": masks,
            "wteT": wteT_s.astype(ml_dtypes.bfloat16),
        })
    return in_maps


def kernel(**inputs):
    from concourse.bass_utils import run_bass_kernel_spmd
    if "nc" not in _BUILT:
        _BUILT["nc"] = _build()
    nc = _BUILT["nc"]
    in_maps = _prep_inputs(inputs)
    res = run_bass_kernel_spmd(nc, in_maps, core_ids=list(range(N_CORES)))
    outs = []
    for b in range(B):
        parts = [res.results[b * TP + r]["logits"]
                 for r in range(TP)]
        outs.append(np.concatenate(parts, axis=1)[:, :V])
    return np.stack(outs, axis=0).astype(np.float32)
